# revision 83
# baseline (speedup 1.0000x reference)
"""Trainium2 Bass kernel for nn_DoubleNet (two GATNet branches + avg-pool + linear).

Strategy (8 NeuronCores):
  - Cores 0-3 run branch A, cores 4-7 run branch B (same SPMD program,
    different input data per core). Within a branch, dst nodes are sharded
    contiguously across the 4 cores.
  - Per GAT layer:
      dense phase: stream x^T (bf16) from the all-gather piece tensors,
        compute z_aug = x @ [W|W@al|W@ar] on the PE (bf16 weights), and stage
        each 128-node chunk into a 512-byte table row: [z0|1|z1|1|z2|1] as
        fp8 in bytes 0:387, el/er as bf16 in bytes 388:400. Two DRAM tables
        ping-pong across layers so the next layer's dense phase can overlap
        the current edge phase.
      edge phase: edges are pre-sorted by dst (host side) and processed in
        chunks of 128, gathered 8 chunks per dma_gather call (the 1024-entry
        SWDGE ring bounds the call size): the full 512B row of each src
        (fp8 z + bf16 el), and the 256B second half-row of each dst (er).
        Gathers issue GLA groups ahead and the attention-weight chain
        (add + leaky_relu on DVE, exp on Act) WLA groups ahead of
        consumption. Per chunk, one of three engine-balanced aggregation
        styles ("o": 3 fused scaled one-hots on DVE + 3 PE matmuls against
        the raw fp8 rows; "v": unscaled one-hot + one broadcast
        tensor_tensor scale on DVE + 1 matmul; "a": one-hot on Pool + 3
        scaled copies on Act + 1 matmul) accumulates messages and softmax
        denominators (the interleaved ones columns) into a per-dst-block
        PSUM tile.
      block tail: denominator reciprocal (DVE), normalize to bf16 (Act),
        PE-transpose, evacuate (DVE), x_next^T = Wl^T @ agg^T (PE), bias
        (Act). Per-core x^T shards all-gather in three pieces; each piece
        triggers the next layer's dense work for the node chunks it covers,
        interleaved into the remaining edge-phase emission.
  - Final layer pools via a host-precomputed gid one-hot matmul; host sums the
    per-core partial pools and applies the output linear (float64).
"""

import sys

sys.path.insert(0, "/opt/trn_rl_repo")

import numpy as np


# ---------------------------------------------------------------------------
# configuration
# ---------------------------------------------------------------------------

class Cfg:
    def __init__(self, N=20000, G=128, H=3, EMB=128, F=128, n_cores=8,
                 neg_slope=0.2, GC=8, BST=4, TB=8, scale_eng="bal"):
        assert F == 128 and EMB == 128 and H == 3
        self.N, self.G, self.H, self.EMB, self.F = N, G, H, EMB, F
        self.n_cores = n_cores
        self.gpb = n_cores // 2            # cores per branch
        assert N % self.gpb == 0
        self.SH = N // self.gpb            # dst nodes per core
        self.NB = -(-self.SH // 128)       # dst blocks per core
        self.NT = -(-N // 128)             # node chunks for dense phase
        self.NTP = self.NT * 128           # padded node count
        self.neg_slope = neg_slope
        self.GC = GC                       # chunks per z-gather call
        self.BST = BST                     # chunks staged per table write
        self.TB = TB                       # chunks per dense x load
        self.scale_eng = scale_eng         # engines for per-head w-scaling
        self.ROW = 512                     # table row bytes (fp8 elements)
        self.dma_scratch = 16384           # SWDGE ring carveout bytes/partition
        self.psz_bufs = 3
        self.psb_bufs = 2
        self.pst_bufs = 1
        self.psx_bufs = 1
        self.GLA = 4                       # gather lookahead (groups)
        self.WLA = 2                       # attention-weight chain lookahead
        self.pat = "ooavooaoooavooaoo"
        self.pat2 = None                   # layer-2 override (no overlap work)
        self.pull = 5                      # dense chunks interleaved per group
        self.cuts = (22, 31)               # all-gather piece block boundaries
        self.zst_bufs = 10
        self.xin_bufs = 4
        self.g_bufs = 8
        self.gs_bufs = 16
        self.stg_split = False
        self.lh_split = False
        self.u_bufs = 16


# ---------------------------------------------------------------------------
# host-side data prep
# ---------------------------------------------------------------------------

def _prep_edges(cfg, src, dst, q):
    """Edges of one core (dst in its shard), dst-sorted, fake rows added."""
    lo = q * cfg.SH
    sel = (dst >= lo) & (dst < lo + cfg.SH)
    es = src[sel].astype(np.int64)
    ed = (dst[sel].astype(np.int64) - lo)
    nfake = cfg.NB * 128 - cfg.SH
    if nfake:
        es = np.concatenate([es, np.zeros(nfake, np.int64)])
        ed = np.concatenate([ed, np.arange(cfg.SH, cfg.NB * 128, dtype=np.int64)])
    order = np.argsort(ed, kind="stable")
    es, ed = es[order], ed[order]
    cnt = np.bincount(ed // 128, minlength=cfg.NB)
    return es, ed, cnt, lo


def _pack_core(cfg, es, ed, lo, nc_b):
    """Build flat (block, chunk, slot) arrays padded to nc_b chunks/block."""
    TOT = int(nc_b.sum())
    zsrc = np.zeros(TOT * 128, np.int16)
    edst = np.zeros(TOT * 128, np.int16)
    dst3 = np.full(TOT * 128, -1.0, np.float32)
    epos = np.searchsorted(ed, np.arange(0, cfg.NB * 128 + 1, 128))
    cum = np.concatenate([[0], np.cumsum(nc_b)]).astype(int)
    for b in range(cfg.NB):
        s, e = epos[b], epos[b + 1]
        n = e - s
        o = int(cum[b]) * 128
        zsrc[o:o + n] = es[s:e]
        # fake rows (local id >= SH) must not use an out-of-range er index
        ei = ed[s:e] + lo
        ei[ed[s:e] >= cfg.SH] = 0
        edst[o:o + n] = ei
        dst3[o:o + n] = (ed[s:e] - b * 128).astype(np.float32)
    # index tiles: flat i -> (partition i%16, col i//16), replicated to 128 rows
    def wrap(a):
        return np.tile(a.reshape(-1, 16).T, (8, 1)).copy()
    # dst3 partition-major: [128, TOT]
    d3 = dst3.reshape(TOT, 128).T.copy()
    return wrap(zsrc), wrap(edst), d3


def _prep_branch_weights(cfg, W1, al1, ar1, b1, Wn, aln, arn, bn, Wl, bl):
    H, EMB = cfg.H, cfg.EMB

    def waug(W, al, ar):
        K = W.shape[0]
        out = np.zeros((K, 390), np.float32)
        out[:, :384] = W
        for h in range(H):
            out[:, 384 + h] = W[:, h * EMB:(h + 1) * EMB] @ al[h]
            out[:, 387 + h] = W[:, h * EMB:(h + 1) * EMB] @ ar[h]
        return out

    wl3 = Wl.reshape(3, 128, EMB).astype(np.float32)
    blp1 = (b1 @ Wl + bl).astype(np.float32)
    blpn = (bn @ Wl + bl).astype(np.float32)
    return waug(W1, al1, ar1), waug(Wn, aln, arn), wl3, blp1, blpn


# ---------------------------------------------------------------------------
# device program
# ---------------------------------------------------------------------------

def build_program(cfg, nc_b, timing_mode=False):
    import concourse.bass as bass
    import concourse.mybir as mybir
    import concourse.tile as tile
    from concourse import bacc

    dt = mybir.dt
    f32 = dt.float32
    bf16 = dt.bfloat16
    f8 = dt.float8e4
    Alu = mybir.AluOpType
    Act = mybir.ActivationFunctionType

    NB, NT, SH, GC, BST, TB = cfg.NB, cfg.NT, cfg.SH, cfg.GC, cfg.BST, cfg.TB
    ROW = cfg.ROW
    TOT = int(nc_b.sum())
    cum = np.concatenate([[0], np.cumsum(nc_b)]).astype(int)
    # block index of each chunk
    blk_of = np.zeros(TOT, np.int64)
    for b in range(NB):
        blk_of[cum[b]:cum[b + 1]] = b
    gpb = cfg.gpb
    groups = [list(range(gpb)), list(range(gpb, 2 * gpb))]

    nc = bacc.Bacc("TRN2", target_bir_lowering=False, debug=False,
                   num_devices=cfg.n_cores,
                   dynamic_dma_scratch_size=cfg.dma_scratch)

    # inputs -----------------------------------------------------------------
    xfull = nc.dram_tensor("xfull", [128, cfg.NTP], bf16, kind="ExternalInput")
    waug1_d = nc.dram_tensor("waug1", [128, 390], bf16, kind="ExternalInput")
    waugn_d = nc.dram_tensor("waugn", [128, 390], bf16, kind="ExternalInput")
    wl3_d = nc.dram_tensor("wl3", [3, 128, 128], bf16, kind="ExternalInput")
    blp1_d = nc.dram_tensor("blp1", [128, 1], f32, kind="ExternalInput")
    blpn_d = nc.dram_tensor("blpn", [128, 1], f32, kind="ExternalInput")
    iota_d = nc.dram_tensor("iota", [128, 128], bf16, kind="ExternalInput")
    ident_d = nc.dram_tensor("ident", [128, 128], bf16, kind="ExternalInput")
    dst3_d = nc.dram_tensor("dst3", [128, TOT], f32, kind="ExternalInput")
    zidx_d = nc.dram_tensor("zidx", [128, TOT * 8], dt.int16, kind="ExternalInput")
    eidx_d = nc.dram_tensor("eidx", [128, TOT * 8], dt.int16, kind="ExternalInput")
    poolw_d = nc.dram_tensor("poolw", [NB, 128, 128], bf16, kind="ExternalInput")
    pool_out = nc.dram_tensor("pool_out", [128, 128], f32, kind="ExternalOutput")

    # internal DRAM ----------------------------------------------------------
    # Two z-tables ping-pong so layer L+1's dense phase (for the SH1 node
    # regions that all-gather mid-edge-phase) can overlap layer L's edge
    # phase without clobbering rows its gathers still read.
    zaug2 = [nc.dram_tensor("zaugA", [cfg.NTP, ROW], f8),
             nc.dram_tensor("zaugB", [cfg.NTP, ROW], f8)]
    # The per-core x^T shard all-gathers in P pieces so the next layer's
    # dense phase can start on each piece as soon as it lands.
    bcuts = [0, *cfg.cuts, NB]
    P = len(bcuts) - 1
    pieces = []                            # (b0, b1, col0, ncols)
    for p in range(P):
        b0, b1 = bcuts[p], bcuts[p + 1]
        col0 = b0 * 128
        ncols = min(b1 * 128, SH) - col0
        pieces.append((b0, b1, col0, ncols))
    xshp = [nc.dram_tensor(f"xsh{p}", [128, pieces[p][3]], bf16)
            for p in range(P)]
    xgathp = [nc.dram_tensor(f"xgath{p}", [gpb, 128, pieces[p][3]], bf16)
              for p in range(P)]

    def do_gather(nc, p):
        xs, xg = xshp[p], xgathp[p]
        if timing_mode:
            for j in range(gpb):
                nc.sync.dma_start(xg.ap()[j], xs.ap())
        else:
            nc.gpsimd.collective_compute(
                "AllGather", mybir.AluOpType.bypass, replica_groups=groups,
                ins=[xs.ap()], outs=[xg.ap()])

    # node-interval -> all-gather piece map (for dense-phase x loads)
    xpieces = []
    for j in range(gpb):
        for p in range(P):
            glo = j * SH + pieces[p][2]
            xpieces.append((glo, glo + pieces[p][3], xgathp[p], j))

    piece_of_block = np.zeros(NB, np.int64)
    for p in range(P):
        piece_of_block[bcuts[p]:bcuts[p + 1]] = p

    with tile.TileContext(nc) as tc:
        cpool = tc.alloc_tile_pool(name="const", bufs=1)
        waug1 = cpool.tile([128, 390], bf16, tag="waug1")
        waugn = cpool.tile([128, 390], bf16, tag="waugn")
        wl3 = cpool.tile([128, 3, 128], bf16, tag="wl3")
        blp1 = cpool.tile([128, 1], f32, tag="blp1")
        iota = cpool.tile([128, 128], bf16, tag="iota")
        ident = cpool.tile([128, 128], bf16, tag="ident")
        dst3 = cpool.tile([128, TOT], f32, tag="dst3")
        zidx = cpool.tile([128, TOT * 8], dt.int16, tag="zidx")
        eidx = cpool.tile([128, TOT * 8], dt.int16, tag="eidx")

        nc.sync.dma_start(waug1[:], waug1_d.ap())
        nc.sync.dma_start(waugn[:], waugn_d.ap())

        xin_pool = tc.alloc_tile_pool(name="xin", bufs=cfg.xin_bufs)
        psz_pool = tc.alloc_tile_pool(name="psz", bufs=cfg.psz_bufs, space="PSUM")
        zst_pool = tc.alloc_tile_pool(name="zst", bufs=cfg.zst_bufs)
        g_pool = tc.alloc_tile_pool(name="g", bufs=cfg.g_bufs)
        r_pool = tc.alloc_tile_pool(name="r", bufs=cfg.g_bufs)
        w_pool = tc.alloc_tile_pool(name="w", bufs=6)
        u_pool = tc.alloc_tile_pool(name="u", bufs=cfg.u_bufs)
        gs_pool = tc.alloc_tile_pool(name="gs", bufs=cfg.gs_bufs)
        psb_pool = tc.alloc_tile_pool(name="psb", bufs=cfg.psb_bufs, space="PSUM")
        s_pool = tc.alloc_tile_pool(name="s", bufs=2)
        a_pool = tc.alloc_tile_pool(name="a", bufs=2)
        if cfg.pst_bufs:
            pst_pool = tc.alloc_tile_pool(name="pst", bufs=cfg.pst_bufs,
                                          space="PSUM")
        at_pool = tc.alloc_tile_pool(name="at", bufs=2)
        psx_pool = tc.alloc_tile_pool(name="psx", bufs=cfg.psx_bufs,
                                      space="PSUM")
        if not cfg.pst_bufs:
            pst_pool = psx_pool        # transposes share the psx banks
        x_pool = tc.alloc_tile_pool(name="x", bufs=2)
        pw_pool = tc.alloc_tile_pool(name="pw", bufs=2)
        pp_pool = tc.alloc_tile_pool(name="pp", bufs=1, space="PSUM")

        ps_pool_acc = pp_pool.tile([128, 128], f32, tag="poolacc")

        # Per-chunk aggregation styles, rotated to balance engines:
        #   "o": 3 fused scaled-one-hots on DVE (is_equal*w) + 3 PE matmuls
        #        against the raw fp8 rows -- cheap on DVE, heavy on PE SEQ.
        #   "v": unscaled one-hot + ONE fused broadcast tensor_tensor scale
        #        on DVE + 1 PE matmul.
        #   "a": unscaled one-hot (Pool) + 3 scaled copies on Act + 1 matmul.
        # Pool otherwise only generates gather descriptors (its in-order
        # sequencer must not block behind data-dependent work).
        _PAT = cfg.pat

        def scale_chunk(eng, c, cl, Gs, Gt, wt):
            if eng == "a":
                for h in range(3):
                    sl = slice(129 * h, 129 * h + 129)
                    nc.scalar.activation(Gs[:, sl], Gt[:, cl, sl].opt(),
                                         Act.Copy,
                                         scale=wt[:, cl, h:h + 1].opt())
            else:
                dst = Gs[:, 0:387].rearrange("p (h c) -> p h c", h=3)
                src = Gt[:, cl, 0:387].rearrange("p (h c) -> p h c", h=3)
                wb = wt[:, cl, :].unsqueeze(-1).broadcast_to([128, 3, 129])
                if eng == "v":
                    nc.vector.tensor_tensor(dst, src, wb, Alu.mult)
                else:
                    nc.gpsimd.tensor_tensor(dst, src, wb, Alu.mult)

        # Dense-phase chunk runs per all-gather piece: a node chunk becomes
        # computable once every piece covering it has landed; chunks fully
        # inside one piece's region go to that piece, stragglers go last.
        assigned = np.full(NT, P - 1, np.int64)
        for p in range(P):
            for j in range(gpb):
                glo = j * SH + pieces[p][2]
                ghi = glo + pieces[p][3]
                lo = -(-glo // 128)
                hi = ghi // 128
                assigned[lo:hi] = p

        def runs_of(p):
            runs, s = [], None
            for t in range(NT):
                if assigned[t] == p and s is None:
                    s = t
                elif assigned[t] != p and s is not None:
                    runs.append((s, t))
                    s = None
            if s is not None:
                runs.append((s, NT))
            return runs

        piece_runs = [runs_of(p) for p in range(P)]

        def load_x(layer, xin, t, tb):
            """Fill xin[:, 0:tb, :] with x^T nodes [t*128, (t+tb)*128),
            reading xfull (layer 0) or the all-gather piece tensors."""
            a, bnd = t * 128, (t + tb) * 128
            flat = xin[:, 0:tb, :].rearrange("p c n -> p (c n)")
            if layer == 0:
                nc.sync.dma_start(flat, xfull.ap()[:, a:bnd])
                return
            for glo, ghi, tens, j in xpieces:
                lo, hi = max(a, glo), min(bnd, ghi)
                if lo < hi:
                    nc.sync.dma_start(flat[:, lo - a:hi - a],
                                      tens.ap()[j][:, lo - glo:hi - glo])

        def dense_gen(layer, runs, in_edge=False):
            """Generator emitting the fp8 z_aug table build for `runs` of
            node chunks; yields after each staged chunk. in_edge: emitted
            interleaved with the edge phase, where Pool is saturated with
            gather descriptor generation -- keep memsets off Pool there."""
            wa = waug1 if layer == 0 else waugn
            tab = zaug2[layer % 2]
            for r0_, r1_ in runs:
                t = r0_
                while t < r1_:
                    tb = min(TB, r1_ - t)
                    xin = xin_pool.tile([128, TB, 128], bf16, tag="xin")
                    load_x(layer, xin, t, tb)
                    s0 = 0
                    while s0 < tb:
                        sb = min(BST, tb - s0)
                        zt = zst_pool.tile([128, BST, ROW], f8, tag="zt")
                        ztb = zt.bitcast(bf16)
                        for s in range(sb):
                            psz = psz_pool.tile([128, 390], f32, tag="psz")
                            nc.tensor.matmul(psz[:], xin[:, s0 + s, :].opt(),
                                             wa[:], start=True, stop=True)
                            nc.gpsimd.memset(zt[:, s, 128:387:129], 1.0)
                            if cfg.stg_split and not in_edge:
                                # both engines stage each chunk (shorter psz
                                # hold): Act heads 0-1, DVE head 2 + el/er
                                nc.scalar.activation(
                                    zt[:, s, 0:258]
                                    .rearrange("p (g c) -> p g c", g=2)
                                    [:, :, 0:128],
                                    psz[:, 0:256]
                                    .rearrange("p (g c) -> p g c", g=2),
                                    Act.Copy)
                                nc.vector.tensor_copy(zt[:, s, 258:386],
                                                      psz[:, 256:384])
                                nc.vector.tensor_copy(ztb[:, s, 194:200],
                                                      psz[:, 384:390])
                                continue_yield = None
                            else:
                                zdst = (zt[:, s, 0:387]
                                        .rearrange("p (g c) -> p g c", g=3)
                                        [:, :, 0:128])
                                zsrc = (psz[:, 0:384]
                                        .rearrange("p (g c) -> p g c", g=3))
                                if (t + s0 + s) % 2:
                                    nc.scalar.activation(zdst, zsrc, Act.Copy)
                                    nc.vector.tensor_copy(ztb[:, s, 194:200],
                                                          psz[:, 384:390])
                                else:
                                    nc.vector.tensor_copy(zdst, zsrc)
                                    nc.scalar.activation(ztb[:, s, 194:200],
                                                         psz[:, 384:390],
                                                         Act.Copy)
                            yield
                        r0 = (t + s0) * 128
                        nc.sync.dma_start(
                            tab.ap()[r0:r0 + sb * 128, :]
                            .rearrange("(c p) z -> p c z", p=128),
                            zt[:, 0:sb, :])
                        s0 += sb
                    t += tb

        for _ in dense_gen(0, [(0, NT)]):
            pass

        # edge-phase-only constants load during the layer-0 dense phase so
        # they don't delay its first x loads on the SP queue
        nc.sync.dma_start(wl3[:], wl3_d.ap().rearrange("k p m -> p k m"))
        nc.sync.dma_start(blp1[:], blp1_d.ap())
        nc.sync.dma_start(iota[:], iota_d.ap())
        nc.sync.dma_start(ident[:], ident_d.ap())
        nc.sync.dma_start(dst3[:], dst3_d.ap())
        nc.sync.dma_start(zidx[:], zidx_d.ap())
        nc.sync.dma_start(eidx[:], eidx_d.ap())

        for layer in range(3):
            zaug = zaug2[layer % 2]
            # next layer's dense work, one generator per landed gather piece,
            # interleaved into this layer's edge phase
            if layer < 2:
                ogens = [[bcuts[p + 1] + 1,
                          dense_gen(layer + 1, piece_runs[p], in_edge=True)]
                         for p in range(P - 1)]
            else:
                ogens = []
            tail_done = [-1]

            # ---------------- edge phase ------------------------------------
            # Software pipeline: gathers issue GLA groups ahead and the
            # attention-weight chain (add/lrelu/exp) WLA groups ahead of the
            # group whose chunks are being consumed, so in-order engine
            # streams never sit at a semaphore for data that could have been
            # requested earlier.
            n_groups = -(-TOT // GC)
            GLA, WLA = cfg.GLA, cfg.WLA
            inflight = {}

            def issue_gathers(g):
                c0 = g * GC
                gsz = min(GC, TOT - c0)
                Gt = g_pool.tile([128, GC, ROW], f8, tag="G")
                R = r_pool.tile([128, GC, 256], f8, tag="R")
                nc.gpsimd.dma_gather(
                    Gt[:, 0:gsz, :], zaug.ap(),
                    zidx[:, 8 * c0: 8 * (c0 + gsz)],
                    num_idxs=gsz * 128, num_idxs_reg=gsz * 128,
                    elem_size=ROW, elem_step=ROW)
                nc.gpsimd.dma_gather(
                    R[:, 0:gsz, :], zaug.ap()[:, 256:512],
                    eidx[:, 8 * c0: 8 * (c0 + gsz)],
                    num_idxs=gsz * 128, num_idxs_reg=gsz * 128,
                    elem_size=256, elem_step=ROW)
                inflight[g] = [Gt, R, None]

            def issue_wt(g):
                c0 = g * GC
                gsz = min(GC, TOT - c0)
                Gt, R, _ = inflight[g]
                Gtb = Gt.bitcast(bf16)       # [128, GC, 256]
                Rb = R.bitcast(bf16)         # [128, GC, 128]
                wt = w_pool.tile([128, GC, 3], f32, tag="wt")
                nc.vector.tensor_tensor(
                    wt[:, 0:gsz, :], Gtb[:, 0:gsz, 194:197],
                    Rb[:, 0:gsz, 69:72], Alu.add)
                nc.vector.scalar_tensor_tensor(
                    wt[:, 0:gsz, :], wt[:, 0:gsz, :], cfg.neg_slope,
                    wt[:, 0:gsz, :], Alu.mult, Alu.max)
                nc.scalar.activation(wt[:, 0:gsz, :], wt[:, 0:gsz, :], Act.Exp)
                inflight[g][2] = wt

            for g in range(min(GLA, n_groups)):
                issue_gathers(g)
            for g in range(min(WLA, n_groups)):
                issue_wt(g)
            psb = None
            for gi in range(n_groups):
                if gi + GLA < n_groups:
                    issue_gathers(gi + GLA)
                if gi + WLA < n_groups:
                    issue_wt(gi + WLA)
                pulled = 0
                for og in ogens:
                    if og[1] is None or tail_done[0] < og[0]:
                        continue
                    while pulled < cfg.pull:
                        if next(og[1], "done") == "done":
                            og[1] = None
                            break
                        pulled += 1
                    if pulled >= cfg.pull:
                        break
                c0 = gi * GC
                gsz = min(GC, TOT - c0)
                Gt, R, wt = inflight.pop(gi)
                for cl in range(gsz):
                    c = c0 + cl
                    b = int(blk_of[c])
                    first = c == int(cum[b])
                    last = c == int(cum[b + 1]) - 1
                    if first:
                        psb = psb_pool.tile([128, 387], f32, tag="psb")
                    eng = cfg.scale_eng
                    if eng == "bal":
                        lp = (cfg.pat2 if layer == 2 and cfg.pat2 else _PAT)
                        eng = lp[c % len(lp)]
                    if eng == "o":
                        if cfg.lh_split:
                            lhs = []
                            for h in range(3):
                                lht = u_pool.tile([128, 128], bf16,
                                                  tag="u", name=f"lh{h}")
                                lhs.append(lht)
                        else:
                            lh3 = gs_pool.tile([128, 3, 128], bf16, tag="Gs")
                            lhs = [lh3[:, h, :].opt() for h in range(3)]
                        for h in range(3):
                            lh = lhs[h][:] if cfg.lh_split else lhs[h]
                            nc.vector.tensor_scalar(
                                lh, iota[:], dst3[:, c:c + 1],
                                wt[:, cl, h:h + 1].opt(),
                                Alu.is_equal, Alu.mult)
                        for h in range(3):
                            sl = slice(129 * h, 129 * h + 129)
                            lh = lhs[h][:] if cfg.lh_split else lhs[h]
                            nc.tensor.matmul(psb[:, sl], lh,
                                             Gt[:, cl, sl].opt(),
                                             start=(first and h == 0),
                                             stop=(last and h == 2))
                    else:
                        u = u_pool.tile([128, 128], bf16, tag="u")
                        if eng == "a":
                            nc.gpsimd.tensor_scalar(u[:], iota[:],
                                                    dst3[:, c:c + 1],
                                                    None, Alu.is_equal)
                        else:
                            nc.vector.tensor_scalar(u[:], iota[:],
                                                    dst3[:, c:c + 1],
                                                    None, Alu.is_equal)
                        Gs = gs_pool.tile([128, 390], bf16, tag="Gs")
                        scale_chunk(eng, c, cl, Gs, Gt, wt)
                        nc.tensor.matmul(psb[:, 0:387], u[:], Gs[:, 0:387],
                                         start=first, stop=last)
                    if last:
                        # ---------------- block tail ------------------------
                        r3 = s_pool.tile([128, 3], f32, tag="r3")
                        nc.vector.reciprocal(r3[:], psb[:, 128:387:129])
                        agg = a_pool.tile([128, 3, 128], bf16, tag="agg")
                        for h in range(3):
                            nc.scalar.activation(
                                agg[:, h, :].opt(),
                                psb[:, 129 * h:129 * h + 128],
                                Act.Copy, scale=r3[:, h:h + 1].opt())
                        aggT = at_pool.tile([128, 3, 128], bf16, tag="aggT")
                        for h in range(3):
                            pst = pst_pool.tile([128, 128], bf16, tag="pst")
                            nc.tensor.transpose(pst[:], agg[:, h, :].opt(),
                                                ident[:])
                            nc.vector.tensor_copy(aggT[:, h, :].opt(), pst[:])
                        bw = min(128, SH - b * 128)
                        psx = psx_pool.tile([128, 128], f32, tag="psx")
                        if layer < 2:
                            for k in range(3):
                                nc.tensor.matmul(psx[:], wl3[:, k, :].opt(),
                                                 aggT[:, k, :].opt(),
                                                 start=(k == 0), stop=(k == 2))
                            xsb = x_pool.tile([128, 128], bf16, tag="xsb")
                            nc.scalar.activation(xsb[:], psx[:], Act.Identity,
                                                 bias=blp1[:])
                            p = int(piece_of_block[b])
                            o = b * 128 - pieces[p][2]
                            nc.sync.dma_start(
                                xshp[p].ap()[:, o:o + bw], xsb[:, 0:bw])
                            if p < P - 1 and b == bcuts[p + 1] - 1:
                                do_gather(nc, p)
                            tail_done[0] = b
                        else:
                            for k in range(3):
                                nc.tensor.matmul(psx[:], aggT[:, k, :].opt(),
                                                 wl3[:, k, :].opt(),
                                                 start=(k == 0), stop=(k == 2))
                            x3 = x_pool.tile([128, 128], bf16, tag="xsb")
                            nc.vector.tensor_copy(x3[:], psx[:])
                            pw = pw_pool.tile([128, 128], bf16, tag="pw")
                            nc.sync.dma_start(pw[:], poolw_d.ap()[b])
                            nc.tensor.matmul(ps_pool_acc[:], pw[:], x3[:],
                                             start=(b == 0), stop=(b == NB - 1))

            for og in ogens:
                if og[1] is not None:
                    for _ in og[1]:
                        pass
            if layer < 2:
                do_gather(nc, P - 1)
                if layer == 0:
                    nc.sync.dma_start(blp1[:], blpn_d.ap())
                for _ in dense_gen(layer + 1, piece_runs[P - 1]):
                    pass

        po = x_pool.tile([128, 128], f32, tag="po")
        nc.vector.tensor_copy(po[:], ps_pool_acc[:])
        nc.sync.dma_start(pool_out.ap(), po[:])

        _rel_pst = (pst_pool,) if cfg.pst_bufs else ()
        for p in (pp_pool, pw_pool, x_pool, psx_pool, at_pool, *_rel_pst,
                  a_pool, s_pool, psb_pool, gs_pool, u_pool, w_pool,
                  r_pool, g_pool, zst_pool, psz_pool, xin_pool, cpool):
            p.release()

    nc.compile()
    return nc


# ---------------------------------------------------------------------------
# top-level kernel
# ---------------------------------------------------------------------------

def _prepare(cfg, inputs):
    """Returns (nc_b, in_maps, host_meta)."""
    import ml_dtypes
    bf = ml_dtypes.bfloat16
    npf = np.asarray
    per_core_edges = []
    nc_b = np.zeros(cfg.NB, np.int64)
    for br, (s, d) in enumerate((("srcA", "dstA"), ("srcB", "dstB"))):
        src = npf(inputs[s]).astype(np.int64)
        dst = npf(inputs[d]).astype(np.int64)
        for q in range(cfg.gpb):
            es, ed, cnt, lo = _prep_edges(cfg, src, dst, q)
            per_core_edges.append((es, ed, lo))
            nc_b = np.maximum(nc_b, -(-cnt // 128))
    in_maps = []
    host_meta = {}
    iota = np.tile(np.arange(128, dtype=bf), (128, 1))
    ident = np.eye(128, dtype=bf)
    for br in range(2):
        sfx = "AB"[br]
        W1 = npf(inputs["W1" + sfx]); al1 = npf(inputs["al1" + sfx])
        ar1 = npf(inputs["ar1" + sfx]); b1 = npf(inputs["b1" + sfx])
        Wn = npf(inputs["Wn" + sfx]); aln = npf(inputs["aln" + sfx])
        arn = npf(inputs["arn" + sfx]); bn = npf(inputs["bn" + sfx])
        Wl = npf(inputs["Wl" + sfx]); bl = npf(inputs["bl" + sfx])
        gid = npf(inputs["gid" + sfx]).astype(np.int64)
        feats = npf(inputs["feats" + sfx]).astype(np.float32)
        waug1, waugn, wl3, blp1, blpn = _prep_branch_weights(
            cfg, W1, al1, ar1, b1, Wn, aln, arn, bn, Wl, bl)
        xfull = np.zeros((128, cfg.NTP), bf)
        xfull[:, :cfg.N] = feats.T.astype(bf)
        host_meta[sfx] = dict(blpn=blpn, gid=gid)
        for q in range(cfg.gpb):
            es, ed, lo = per_core_edges[br * cfg.gpb + q]
            zidx, eidx, dst3 = _pack_core(cfg, es, ed, lo, nc_b)
            poolw = np.zeros((cfg.NB, 128, 128), bf)
            for b in range(cfg.NB):
                for i in range(min(128, cfg.SH - b * 128)):
                    n = lo + b * 128 + i
                    if n < cfg.N:
                        poolw[b, i, gid[n]] = 1.0
            in_maps.append({
                "xfull": xfull,
                "waug1": waug1.astype(bf), "waugn": waugn.astype(bf),
                "wl3": wl3.astype(bf), "blp1": blp1.reshape(128, 1),
                "blpn": blpn.reshape(128, 1),
                "iota": iota, "ident": ident,
                "dst3": dst3, "zidx": zidx, "eidx": eidx, "poolw": poolw,
            })
    return nc_b, in_maps, host_meta


def _finalize(cfg, inputs, host_meta, pool_outs):
    """pool_outs: list of 8 [128,128] arrays -> full output [G,1] float64."""
    out = {}
    for br in range(2):
        sfx = "AB"[br]
        total = np.zeros((128, 128), np.float64)
        for q in range(cfg.gpb):
            total += pool_outs[br * cfg.gpb + q].astype(np.float64)
        gid = host_meta[sfx]["gid"]
        cnt = np.bincount(gid, minlength=128).astype(np.float64)
        total += cnt[:, None] * host_meta[sfx]["blpn"].astype(np.float64)[None, :]
        out[sfx] = (total / np.maximum(cnt[:, None], 1.0))[:cfg.G]
    cat = np.concatenate([out["A"], out["B"]], axis=1)
    Wo = np.asarray(inputs["Wo"]).astype(np.float64)
    bo = np.asarray(inputs["bo"]).astype(np.float64)
    return (cat @ Wo + bo).astype(np.float64)


_CACHE = {}


def kernel(**inputs):
    cfg = Cfg(N=inputs["featsA"].shape[0], G=128)
    nc_b, in_maps, host_meta = _prepare(cfg, inputs)
    key = ("prog", tuple(nc_b.tolist()))
    if key not in _CACHE:
        _CACHE[key] = build_program(cfg, nc_b)
    nc = _CACHE[key]
    from concourse.bass_utils import run_bass_kernel_spmd
    res = run_bass_kernel_spmd(nc, in_maps, list(range(cfg.n_cores)))
    pool_outs = [r["pool_out"] for r in res.results]
    return _finalize(cfg, inputs, host_meta, pool_outs)


# revision 84
# speedup vs baseline: 1.0012x; 1.0012x over previous
"""Trainium2 Bass kernel for nn_DoubleNet (two GATNet branches + avg-pool + linear).

Strategy (8 NeuronCores):
  - Cores 0-3 run branch A, cores 4-7 run branch B (same SPMD program,
    different input data per core). Within a branch, dst nodes are sharded
    contiguously across the 4 cores.
  - Per GAT layer:
      dense phase: stream x^T (bf16) from the all-gather piece tensors,
        compute z_aug = x @ [W|W@al|W@ar] on the PE (bf16 weights), and stage
        each 128-node chunk into a 512-byte table row: [z0|1|z1|1|z2|1] as
        fp8 in bytes 0:387, el/er as bf16 in bytes 388:400. Two DRAM tables
        ping-pong across layers so the next layer's dense phase can overlap
        the current edge phase.
      edge phase: edges are pre-sorted by dst (host side) and processed in
        chunks of 128, gathered 8 chunks per dma_gather call (the 1024-entry
        SWDGE ring bounds the call size): the full 512B row of each src
        (fp8 z + bf16 el), and the 256B second half-row of each dst (er).
        Gathers issue GLA groups ahead and the attention-weight chain
        (add + leaky_relu on DVE, exp on Act) WLA groups ahead of
        consumption. Per chunk, one of three engine-balanced aggregation
        styles ("o": 3 fused scaled one-hots on DVE + 3 PE matmuls against
        the raw fp8 rows; "v": unscaled one-hot + one broadcast
        tensor_tensor scale on DVE + 1 matmul; "a": one-hot on Pool + 3
        scaled copies on Act + 1 matmul) accumulates messages and softmax
        denominators (the interleaved ones columns) into a per-dst-block
        PSUM tile.
      block tail: denominator reciprocal (DVE), normalize to bf16 (Act),
        PE-transpose, evacuate (DVE), x_next^T = Wl^T @ agg^T (PE), bias
        (Act). Per-core x^T shards all-gather in three pieces; each piece
        triggers the next layer's dense work for the node chunks it covers,
        interleaved into the remaining edge-phase emission.
  - Final layer pools via a host-precomputed gid one-hot matmul; host sums the
    per-core partial pools and applies the output linear (float64).
"""

import sys

sys.path.insert(0, "/opt/trn_rl_repo")

import numpy as np


# ---------------------------------------------------------------------------
# configuration
# ---------------------------------------------------------------------------

class Cfg:
    def __init__(self, N=20000, G=128, H=3, EMB=128, F=128, n_cores=8,
                 neg_slope=0.2, GC=8, BST=4, TB=8, scale_eng="bal"):
        assert F == 128 and EMB == 128 and H == 3
        self.N, self.G, self.H, self.EMB, self.F = N, G, H, EMB, F
        self.n_cores = n_cores
        self.gpb = n_cores // 2            # cores per branch
        assert N % self.gpb == 0
        self.SH = N // self.gpb            # dst nodes per core
        self.NB = -(-self.SH // 128)       # dst blocks per core
        self.NT = -(-N // 128)             # node chunks for dense phase
        self.NTP = self.NT * 128           # padded node count
        self.neg_slope = neg_slope
        self.GC = GC                       # chunks per z-gather call
        self.BST = BST                     # chunks staged per table write
        self.TB = TB                       # chunks per dense x load
        self.scale_eng = scale_eng         # engines for per-head w-scaling
        self.ROW = 512                     # table row bytes (fp8 elements)
        self.dma_scratch = 16384           # SWDGE ring carveout bytes/partition
        self.psz_bufs = 3
        self.psb_bufs = 2
        self.pst_bufs = 1
        self.psx_bufs = 1
        self.GLA = 4                       # gather lookahead (groups)
        self.WLA = 2                       # attention-weight chain lookahead
        self.pat = "ooavooaoooavooaoo"
        self.pat2 = None                   # layer-2 override (no overlap work)
        self.pull = 5                      # dense chunks interleaved per group
        self.cuts = (22, 31)               # all-gather piece block boundaries
        self.zst_bufs = 12
        self.xin_bufs = 4
        self.g_bufs = 8
        self.gs_bufs = 16
        self.stg_split = False
        self.lh_split = False
        self.u_bufs = 16


# ---------------------------------------------------------------------------
# host-side data prep
# ---------------------------------------------------------------------------

def _prep_edges(cfg, src, dst, q):
    """Edges of one core (dst in its shard), dst-sorted, fake rows added."""
    lo = q * cfg.SH
    sel = (dst >= lo) & (dst < lo + cfg.SH)
    es = src[sel].astype(np.int64)
    ed = (dst[sel].astype(np.int64) - lo)
    nfake = cfg.NB * 128 - cfg.SH
    if nfake:
        es = np.concatenate([es, np.zeros(nfake, np.int64)])
        ed = np.concatenate([ed, np.arange(cfg.SH, cfg.NB * 128, dtype=np.int64)])
    order = np.argsort(ed, kind="stable")
    es, ed = es[order], ed[order]
    cnt = np.bincount(ed // 128, minlength=cfg.NB)
    return es, ed, cnt, lo


def _pack_core(cfg, es, ed, lo, nc_b):
    """Build flat (block, chunk, slot) arrays padded to nc_b chunks/block."""
    TOT = int(nc_b.sum())
    zsrc = np.zeros(TOT * 128, np.int16)
    edst = np.zeros(TOT * 128, np.int16)
    dst3 = np.full(TOT * 128, -1.0, np.float32)
    epos = np.searchsorted(ed, np.arange(0, cfg.NB * 128 + 1, 128))
    cum = np.concatenate([[0], np.cumsum(nc_b)]).astype(int)
    for b in range(cfg.NB):
        s, e = epos[b], epos[b + 1]
        n = e - s
        o = int(cum[b]) * 128
        zsrc[o:o + n] = es[s:e]
        # fake rows (local id >= SH) must not use an out-of-range er index
        ei = ed[s:e] + lo
        ei[ed[s:e] >= cfg.SH] = 0
        edst[o:o + n] = ei
        dst3[o:o + n] = (ed[s:e] - b * 128).astype(np.float32)
    # index tiles: flat i -> (partition i%16, col i//16), replicated to 128 rows
    def wrap(a):
        return np.tile(a.reshape(-1, 16).T, (8, 1)).copy()
    # dst3 partition-major: [128, TOT]
    d3 = dst3.reshape(TOT, 128).T.copy()
    return wrap(zsrc), wrap(edst), d3


def _prep_branch_weights(cfg, W1, al1, ar1, b1, Wn, aln, arn, bn, Wl, bl):
    H, EMB = cfg.H, cfg.EMB

    def waug(W, al, ar):
        K = W.shape[0]
        out = np.zeros((K, 390), np.float32)
        out[:, :384] = W
        for h in range(H):
            out[:, 384 + h] = W[:, h * EMB:(h + 1) * EMB] @ al[h]
            out[:, 387 + h] = W[:, h * EMB:(h + 1) * EMB] @ ar[h]
        return out

    wl3 = Wl.reshape(3, 128, EMB).astype(np.float32)
    blp1 = (b1 @ Wl + bl).astype(np.float32)
    blpn = (bn @ Wl + bl).astype(np.float32)
    return waug(W1, al1, ar1), waug(Wn, aln, arn), wl3, blp1, blpn


# ---------------------------------------------------------------------------
# device program
# ---------------------------------------------------------------------------

def build_program(cfg, nc_b, timing_mode=False):
    import concourse.bass as bass
    import concourse.mybir as mybir
    import concourse.tile as tile
    from concourse import bacc

    dt = mybir.dt
    f32 = dt.float32
    bf16 = dt.bfloat16
    f8 = dt.float8e4
    Alu = mybir.AluOpType
    Act = mybir.ActivationFunctionType

    NB, NT, SH, GC, BST, TB = cfg.NB, cfg.NT, cfg.SH, cfg.GC, cfg.BST, cfg.TB
    ROW = cfg.ROW
    TOT = int(nc_b.sum())
    cum = np.concatenate([[0], np.cumsum(nc_b)]).astype(int)
    # block index of each chunk
    blk_of = np.zeros(TOT, np.int64)
    for b in range(NB):
        blk_of[cum[b]:cum[b + 1]] = b
    gpb = cfg.gpb
    groups = [list(range(gpb)), list(range(gpb, 2 * gpb))]

    nc = bacc.Bacc("TRN2", target_bir_lowering=False, debug=False,
                   num_devices=cfg.n_cores,
                   dynamic_dma_scratch_size=cfg.dma_scratch)

    # inputs -----------------------------------------------------------------
    xfull = nc.dram_tensor("xfull", [128, cfg.NTP], bf16, kind="ExternalInput")
    waug1_d = nc.dram_tensor("waug1", [128, 390], bf16, kind="ExternalInput")
    waugn_d = nc.dram_tensor("waugn", [128, 390], bf16, kind="ExternalInput")
    wl3_d = nc.dram_tensor("wl3", [3, 128, 128], bf16, kind="ExternalInput")
    blp1_d = nc.dram_tensor("blp1", [128, 1], f32, kind="ExternalInput")
    blpn_d = nc.dram_tensor("blpn", [128, 1], f32, kind="ExternalInput")
    iota_d = nc.dram_tensor("iota", [128, 128], bf16, kind="ExternalInput")
    ident_d = nc.dram_tensor("ident", [128, 128], bf16, kind="ExternalInput")
    dst3_d = nc.dram_tensor("dst3", [128, TOT], f32, kind="ExternalInput")
    zidx_d = nc.dram_tensor("zidx", [128, TOT * 8], dt.int16, kind="ExternalInput")
    eidx_d = nc.dram_tensor("eidx", [128, TOT * 8], dt.int16, kind="ExternalInput")
    poolw_d = nc.dram_tensor("poolw", [NB, 128, 128], bf16, kind="ExternalInput")
    pool_out = nc.dram_tensor("pool_out", [128, 128], f32, kind="ExternalOutput")

    # internal DRAM ----------------------------------------------------------
    # Two z-tables ping-pong so layer L+1's dense phase (for the SH1 node
    # regions that all-gather mid-edge-phase) can overlap layer L's edge
    # phase without clobbering rows its gathers still read.
    zaug2 = [nc.dram_tensor("zaugA", [cfg.NTP, ROW], f8),
             nc.dram_tensor("zaugB", [cfg.NTP, ROW], f8)]
    # The per-core x^T shard all-gathers in P pieces so the next layer's
    # dense phase can start on each piece as soon as it lands.
    bcuts = [0, *cfg.cuts, NB]
    P = len(bcuts) - 1
    pieces = []                            # (b0, b1, col0, ncols)
    for p in range(P):
        b0, b1 = bcuts[p], bcuts[p + 1]
        col0 = b0 * 128
        ncols = min(b1 * 128, SH) - col0
        pieces.append((b0, b1, col0, ncols))
    xshp = [nc.dram_tensor(f"xsh{p}", [128, pieces[p][3]], bf16)
            for p in range(P)]
    xgathp = [nc.dram_tensor(f"xgath{p}", [gpb, 128, pieces[p][3]], bf16)
              for p in range(P)]

    def do_gather(nc, p):
        xs, xg = xshp[p], xgathp[p]
        if timing_mode:
            for j in range(gpb):
                nc.sync.dma_start(xg.ap()[j], xs.ap())
        else:
            nc.gpsimd.collective_compute(
                "AllGather", mybir.AluOpType.bypass, replica_groups=groups,
                ins=[xs.ap()], outs=[xg.ap()])

    # node-interval -> all-gather piece map (for dense-phase x loads)
    xpieces = []
    for j in range(gpb):
        for p in range(P):
            glo = j * SH + pieces[p][2]
            xpieces.append((glo, glo + pieces[p][3], xgathp[p], j))

    piece_of_block = np.zeros(NB, np.int64)
    for p in range(P):
        piece_of_block[bcuts[p]:bcuts[p + 1]] = p

    with tile.TileContext(nc) as tc:
        cpool = tc.alloc_tile_pool(name="const", bufs=1)
        waug1 = cpool.tile([128, 390], bf16, tag="waug1")
        waugn = cpool.tile([128, 390], bf16, tag="waugn")
        wl3 = cpool.tile([128, 3, 128], bf16, tag="wl3")
        blp1 = cpool.tile([128, 1], f32, tag="blp1")
        iota = cpool.tile([128, 128], bf16, tag="iota")
        ident = cpool.tile([128, 128], bf16, tag="ident")
        dst3 = cpool.tile([128, TOT], f32, tag="dst3")
        zidx = cpool.tile([128, TOT * 8], dt.int16, tag="zidx")
        eidx = cpool.tile([128, TOT * 8], dt.int16, tag="eidx")

        nc.sync.dma_start(waug1[:], waug1_d.ap())
        nc.sync.dma_start(waugn[:], waugn_d.ap())

        xin_pool = tc.alloc_tile_pool(name="xin", bufs=cfg.xin_bufs)
        psz_pool = tc.alloc_tile_pool(name="psz", bufs=cfg.psz_bufs, space="PSUM")
        zst_pool = tc.alloc_tile_pool(name="zst", bufs=cfg.zst_bufs)
        g_pool = tc.alloc_tile_pool(name="g", bufs=cfg.g_bufs)
        r_pool = tc.alloc_tile_pool(name="r", bufs=cfg.g_bufs)
        w_pool = tc.alloc_tile_pool(name="w", bufs=6)
        u_pool = tc.alloc_tile_pool(name="u", bufs=cfg.u_bufs)
        gs_pool = tc.alloc_tile_pool(name="gs", bufs=cfg.gs_bufs)
        psb_pool = tc.alloc_tile_pool(name="psb", bufs=cfg.psb_bufs, space="PSUM")
        s_pool = tc.alloc_tile_pool(name="s", bufs=2)
        a_pool = tc.alloc_tile_pool(name="a", bufs=2)
        if cfg.pst_bufs:
            pst_pool = tc.alloc_tile_pool(name="pst", bufs=cfg.pst_bufs,
                                          space="PSUM")
        at_pool = tc.alloc_tile_pool(name="at", bufs=2)
        psx_pool = tc.alloc_tile_pool(name="psx", bufs=cfg.psx_bufs,
                                      space="PSUM")
        if not cfg.pst_bufs:
            pst_pool = psx_pool        # transposes share the psx banks
        x_pool = tc.alloc_tile_pool(name="x", bufs=2)
        pw_pool = tc.alloc_tile_pool(name="pw", bufs=2)
        pp_pool = tc.alloc_tile_pool(name="pp", bufs=1, space="PSUM")

        ps_pool_acc = pp_pool.tile([128, 128], f32, tag="poolacc")

        # Per-chunk aggregation styles, rotated to balance engines:
        #   "o": 3 fused scaled-one-hots on DVE (is_equal*w) + 3 PE matmuls
        #        against the raw fp8 rows -- cheap on DVE, heavy on PE SEQ.
        #   "v": unscaled one-hot + ONE fused broadcast tensor_tensor scale
        #        on DVE + 1 PE matmul.
        #   "a": unscaled one-hot (Pool) + 3 scaled copies on Act + 1 matmul.
        # Pool otherwise only generates gather descriptors (its in-order
        # sequencer must not block behind data-dependent work).
        _PAT = cfg.pat

        def scale_chunk(eng, c, cl, Gs, Gt, wt):
            if eng == "a":
                for h in range(3):
                    sl = slice(129 * h, 129 * h + 129)
                    nc.scalar.activation(Gs[:, sl], Gt[:, cl, sl].opt(),
                                         Act.Copy,
                                         scale=wt[:, cl, h:h + 1].opt())
            else:
                dst = Gs[:, 0:387].rearrange("p (h c) -> p h c", h=3)
                src = Gt[:, cl, 0:387].rearrange("p (h c) -> p h c", h=3)
                wb = wt[:, cl, :].unsqueeze(-1).broadcast_to([128, 3, 129])
                if eng == "v":
                    nc.vector.tensor_tensor(dst, src, wb, Alu.mult)
                else:
                    nc.gpsimd.tensor_tensor(dst, src, wb, Alu.mult)

        # Dense-phase chunk runs per all-gather piece: a node chunk becomes
        # computable once every piece covering it has landed; chunks fully
        # inside one piece's region go to that piece, stragglers go last.
        assigned = np.full(NT, P - 1, np.int64)
        for p in range(P):
            for j in range(gpb):
                glo = j * SH + pieces[p][2]
                ghi = glo + pieces[p][3]
                lo = -(-glo // 128)
                hi = ghi // 128
                assigned[lo:hi] = p

        def runs_of(p):
            runs, s = [], None
            for t in range(NT):
                if assigned[t] == p and s is None:
                    s = t
                elif assigned[t] != p and s is not None:
                    runs.append((s, t))
                    s = None
            if s is not None:
                runs.append((s, NT))
            return runs

        piece_runs = [runs_of(p) for p in range(P)]

        def load_x(layer, xin, t, tb):
            """Fill xin[:, 0:tb, :] with x^T nodes [t*128, (t+tb)*128),
            reading xfull (layer 0) or the all-gather piece tensors."""
            a, bnd = t * 128, (t + tb) * 128
            flat = xin[:, 0:tb, :].rearrange("p c n -> p (c n)")
            if layer == 0:
                nc.sync.dma_start(flat, xfull.ap()[:, a:bnd])
                return
            for glo, ghi, tens, j in xpieces:
                lo, hi = max(a, glo), min(bnd, ghi)
                if lo < hi:
                    nc.sync.dma_start(flat[:, lo - a:hi - a],
                                      tens.ap()[j][:, lo - glo:hi - glo])

        def dense_gen(layer, runs, in_edge=False):
            """Generator emitting the fp8 z_aug table build for `runs` of
            node chunks; yields after each staged chunk. in_edge: emitted
            interleaved with the edge phase, where Pool is saturated with
            gather descriptor generation -- keep memsets off Pool there."""
            wa = waug1 if layer == 0 else waugn
            tab = zaug2[layer % 2]
            for r0_, r1_ in runs:
                t = r0_
                while t < r1_:
                    tb = min(TB, r1_ - t)
                    xin = xin_pool.tile([128, TB, 128], bf16, tag="xin")
                    load_x(layer, xin, t, tb)
                    s0 = 0
                    while s0 < tb:
                        sb = min(BST, tb - s0)
                        zt = zst_pool.tile([128, BST, ROW], f8, tag="zt")
                        ztb = zt.bitcast(bf16)
                        for s in range(sb):
                            psz = psz_pool.tile([128, 390], f32, tag="psz")
                            nc.tensor.matmul(psz[:], xin[:, s0 + s, :].opt(),
                                             wa[:], start=True, stop=True)
                            nc.gpsimd.memset(zt[:, s, 128:387:129], 1.0)
                            if cfg.stg_split and not in_edge:
                                # both engines stage each chunk (shorter psz
                                # hold): Act heads 0-1, DVE head 2 + el/er
                                nc.scalar.activation(
                                    zt[:, s, 0:258]
                                    .rearrange("p (g c) -> p g c", g=2)
                                    [:, :, 0:128],
                                    psz[:, 0:256]
                                    .rearrange("p (g c) -> p g c", g=2),
                                    Act.Copy)
                                nc.vector.tensor_copy(zt[:, s, 258:386],
                                                      psz[:, 256:384])
                                nc.vector.tensor_copy(ztb[:, s, 194:200],
                                                      psz[:, 384:390])
                                continue_yield = None
                            else:
                                zdst = (zt[:, s, 0:387]
                                        .rearrange("p (g c) -> p g c", g=3)
                                        [:, :, 0:128])
                                zsrc = (psz[:, 0:384]
                                        .rearrange("p (g c) -> p g c", g=3))
                                if (t + s0 + s) % 2:
                                    nc.scalar.activation(zdst, zsrc, Act.Copy)
                                    nc.vector.tensor_copy(ztb[:, s, 194:200],
                                                          psz[:, 384:390])
                                else:
                                    nc.vector.tensor_copy(zdst, zsrc)
                                    nc.scalar.activation(ztb[:, s, 194:200],
                                                         psz[:, 384:390],
                                                         Act.Copy)
                            yield
                        r0 = (t + s0) * 128
                        nc.sync.dma_start(
                            tab.ap()[r0:r0 + sb * 128, :]
                            .rearrange("(c p) z -> p c z", p=128),
                            zt[:, 0:sb, :])
                        s0 += sb
                    t += tb

        for _ in dense_gen(0, [(0, NT)]):
            pass

        # edge-phase-only constants load during the layer-0 dense phase so
        # they don't delay its first x loads on the SP queue
        nc.sync.dma_start(wl3[:], wl3_d.ap().rearrange("k p m -> p k m"))
        nc.sync.dma_start(blp1[:], blp1_d.ap())
        nc.sync.dma_start(iota[:], iota_d.ap())
        nc.sync.dma_start(ident[:], ident_d.ap())
        nc.sync.dma_start(dst3[:], dst3_d.ap())
        nc.sync.dma_start(zidx[:], zidx_d.ap())
        nc.sync.dma_start(eidx[:], eidx_d.ap())

        for layer in range(3):
            zaug = zaug2[layer % 2]
            # next layer's dense work, one generator per landed gather piece,
            # interleaved into this layer's edge phase
            if layer < 2:
                ogens = [[bcuts[p + 1] + 1,
                          dense_gen(layer + 1, piece_runs[p], in_edge=True)]
                         for p in range(P - 1)]
            else:
                ogens = []
            tail_done = [-1]

            # ---------------- edge phase ------------------------------------
            # Software pipeline: gathers issue GLA groups ahead and the
            # attention-weight chain (add/lrelu/exp) WLA groups ahead of the
            # group whose chunks are being consumed, so in-order engine
            # streams never sit at a semaphore for data that could have been
            # requested earlier.
            n_groups = -(-TOT // GC)
            GLA, WLA = cfg.GLA, cfg.WLA
            inflight = {}

            def issue_gathers(g):
                c0 = g * GC
                gsz = min(GC, TOT - c0)
                Gt = g_pool.tile([128, GC, ROW], f8, tag="G")
                R = r_pool.tile([128, GC, 256], f8, tag="R")
                nc.gpsimd.dma_gather(
                    Gt[:, 0:gsz, :], zaug.ap(),
                    zidx[:, 8 * c0: 8 * (c0 + gsz)],
                    num_idxs=gsz * 128, num_idxs_reg=gsz * 128,
                    elem_size=ROW, elem_step=ROW)
                nc.gpsimd.dma_gather(
                    R[:, 0:gsz, :], zaug.ap()[:, 256:512],
                    eidx[:, 8 * c0: 8 * (c0 + gsz)],
                    num_idxs=gsz * 128, num_idxs_reg=gsz * 128,
                    elem_size=256, elem_step=ROW)
                inflight[g] = [Gt, R, None]

            def issue_wt(g):
                c0 = g * GC
                gsz = min(GC, TOT - c0)
                Gt, R, _ = inflight[g]
                Gtb = Gt.bitcast(bf16)       # [128, GC, 256]
                Rb = R.bitcast(bf16)         # [128, GC, 128]
                wt = w_pool.tile([128, GC, 3], f32, tag="wt")
                nc.vector.tensor_tensor(
                    wt[:, 0:gsz, :], Gtb[:, 0:gsz, 194:197],
                    Rb[:, 0:gsz, 69:72], Alu.add)
                nc.vector.scalar_tensor_tensor(
                    wt[:, 0:gsz, :], wt[:, 0:gsz, :], cfg.neg_slope,
                    wt[:, 0:gsz, :], Alu.mult, Alu.max)
                nc.scalar.activation(wt[:, 0:gsz, :], wt[:, 0:gsz, :], Act.Exp)
                inflight[g][2] = wt

            for g in range(min(GLA, n_groups)):
                issue_gathers(g)
            for g in range(min(WLA, n_groups)):
                issue_wt(g)
            psb = None
            for gi in range(n_groups):
                if gi + GLA < n_groups:
                    issue_gathers(gi + GLA)
                if gi + WLA < n_groups:
                    issue_wt(gi + WLA)
                pulled = 0
                for og in ogens:
                    if og[1] is None or tail_done[0] < og[0]:
                        continue
                    while pulled < cfg.pull:
                        if next(og[1], "done") == "done":
                            og[1] = None
                            break
                        pulled += 1
                    if pulled >= cfg.pull:
                        break
                c0 = gi * GC
                gsz = min(GC, TOT - c0)
                Gt, R, wt = inflight.pop(gi)
                for cl in range(gsz):
                    c = c0 + cl
                    b = int(blk_of[c])
                    first = c == int(cum[b])
                    last = c == int(cum[b + 1]) - 1
                    if first:
                        psb = psb_pool.tile([128, 387], f32, tag="psb")
                    eng = cfg.scale_eng
                    if eng == "bal":
                        lp = (cfg.pat2 if layer == 2 and cfg.pat2 else _PAT)
                        eng = lp[c % len(lp)]
                    if eng == "o":
                        if cfg.lh_split:
                            lhs = []
                            for h in range(3):
                                lht = u_pool.tile([128, 128], bf16,
                                                  tag="u", name=f"lh{h}")
                                lhs.append(lht)
                        else:
                            lh3 = gs_pool.tile([128, 3, 128], bf16, tag="Gs")
                            lhs = [lh3[:, h, :].opt() for h in range(3)]
                        for h in range(3):
                            lh = lhs[h][:] if cfg.lh_split else lhs[h]
                            nc.vector.tensor_scalar(
                                lh, iota[:], dst3[:, c:c + 1],
                                wt[:, cl, h:h + 1].opt(),
                                Alu.is_equal, Alu.mult)
                        for h in range(3):
                            sl = slice(129 * h, 129 * h + 129)
                            lh = lhs[h][:] if cfg.lh_split else lhs[h]
                            nc.tensor.matmul(psb[:, sl], lh,
                                             Gt[:, cl, sl].opt(),
                                             start=(first and h == 0),
                                             stop=(last and h == 2))
                    else:
                        u = u_pool.tile([128, 128], bf16, tag="u")
                        if eng == "a":
                            nc.gpsimd.tensor_scalar(u[:], iota[:],
                                                    dst3[:, c:c + 1],
                                                    None, Alu.is_equal)
                        else:
                            nc.vector.tensor_scalar(u[:], iota[:],
                                                    dst3[:, c:c + 1],
                                                    None, Alu.is_equal)
                        Gs = gs_pool.tile([128, 390], bf16, tag="Gs")
                        scale_chunk(eng, c, cl, Gs, Gt, wt)
                        nc.tensor.matmul(psb[:, 0:387], u[:], Gs[:, 0:387],
                                         start=first, stop=last)
                    if last:
                        # ---------------- block tail ------------------------
                        r3 = s_pool.tile([128, 3], f32, tag="r3")
                        nc.vector.reciprocal(r3[:], psb[:, 128:387:129])
                        agg = a_pool.tile([128, 3, 128], bf16, tag="agg")
                        for h in range(3):
                            nc.scalar.activation(
                                agg[:, h, :].opt(),
                                psb[:, 129 * h:129 * h + 128],
                                Act.Copy, scale=r3[:, h:h + 1].opt())
                        aggT = at_pool.tile([128, 3, 128], bf16, tag="aggT")
                        for h in range(3):
                            pst = pst_pool.tile([128, 128], bf16, tag="pst")
                            nc.tensor.transpose(pst[:], agg[:, h, :].opt(),
                                                ident[:])
                            nc.vector.tensor_copy(aggT[:, h, :].opt(), pst[:])
                        bw = min(128, SH - b * 128)
                        psx = psx_pool.tile([128, 128], f32, tag="psx")
                        if layer < 2:
                            for k in range(3):
                                nc.tensor.matmul(psx[:], wl3[:, k, :].opt(),
                                                 aggT[:, k, :].opt(),
                                                 start=(k == 0), stop=(k == 2))
                            xsb = x_pool.tile([128, 128], bf16, tag="xsb")
                            nc.scalar.activation(xsb[:], psx[:], Act.Identity,
                                                 bias=blp1[:])
                            p = int(piece_of_block[b])
                            o = b * 128 - pieces[p][2]
                            nc.sync.dma_start(
                                xshp[p].ap()[:, o:o + bw], xsb[:, 0:bw])
                            if p < P - 1 and b == bcuts[p + 1] - 1:
                                do_gather(nc, p)
                            tail_done[0] = b
                        else:
                            for k in range(3):
                                nc.tensor.matmul(psx[:], aggT[:, k, :].opt(),
                                                 wl3[:, k, :].opt(),
                                                 start=(k == 0), stop=(k == 2))
                            x3 = x_pool.tile([128, 128], bf16, tag="xsb")
                            nc.vector.tensor_copy(x3[:], psx[:])
                            pw = pw_pool.tile([128, 128], bf16, tag="pw")
                            nc.sync.dma_start(pw[:], poolw_d.ap()[b])
                            nc.tensor.matmul(ps_pool_acc[:], pw[:], x3[:],
                                             start=(b == 0), stop=(b == NB - 1))

            for og in ogens:
                if og[1] is not None:
                    for _ in og[1]:
                        pass
            if layer < 2:
                do_gather(nc, P - 1)
                if layer == 0:
                    nc.sync.dma_start(blp1[:], blpn_d.ap())
                for _ in dense_gen(layer + 1, piece_runs[P - 1]):
                    pass

        po = x_pool.tile([128, 128], f32, tag="po")
        nc.vector.tensor_copy(po[:], ps_pool_acc[:])
        nc.sync.dma_start(pool_out.ap(), po[:])

        _rel_pst = (pst_pool,) if cfg.pst_bufs else ()
        for p in (pp_pool, pw_pool, x_pool, psx_pool, at_pool, *_rel_pst,
                  a_pool, s_pool, psb_pool, gs_pool, u_pool, w_pool,
                  r_pool, g_pool, zst_pool, psz_pool, xin_pool, cpool):
            p.release()

    nc.compile()
    return nc


# ---------------------------------------------------------------------------
# top-level kernel
# ---------------------------------------------------------------------------

def _prepare(cfg, inputs):
    """Returns (nc_b, in_maps, host_meta)."""
    import ml_dtypes
    bf = ml_dtypes.bfloat16
    npf = np.asarray
    per_core_edges = []
    nc_b = np.zeros(cfg.NB, np.int64)
    for br, (s, d) in enumerate((("srcA", "dstA"), ("srcB", "dstB"))):
        src = npf(inputs[s]).astype(np.int64)
        dst = npf(inputs[d]).astype(np.int64)
        for q in range(cfg.gpb):
            es, ed, cnt, lo = _prep_edges(cfg, src, dst, q)
            per_core_edges.append((es, ed, lo))
            nc_b = np.maximum(nc_b, -(-cnt // 128))
    in_maps = []
    host_meta = {}
    iota = np.tile(np.arange(128, dtype=bf), (128, 1))
    ident = np.eye(128, dtype=bf)
    for br in range(2):
        sfx = "AB"[br]
        W1 = npf(inputs["W1" + sfx]); al1 = npf(inputs["al1" + sfx])
        ar1 = npf(inputs["ar1" + sfx]); b1 = npf(inputs["b1" + sfx])
        Wn = npf(inputs["Wn" + sfx]); aln = npf(inputs["aln" + sfx])
        arn = npf(inputs["arn" + sfx]); bn = npf(inputs["bn" + sfx])
        Wl = npf(inputs["Wl" + sfx]); bl = npf(inputs["bl" + sfx])
        gid = npf(inputs["gid" + sfx]).astype(np.int64)
        feats = npf(inputs["feats" + sfx]).astype(np.float32)
        waug1, waugn, wl3, blp1, blpn = _prep_branch_weights(
            cfg, W1, al1, ar1, b1, Wn, aln, arn, bn, Wl, bl)
        xfull = np.zeros((128, cfg.NTP), bf)
        xfull[:, :cfg.N] = feats.T.astype(bf)
        host_meta[sfx] = dict(blpn=blpn, gid=gid)
        for q in range(cfg.gpb):
            es, ed, lo = per_core_edges[br * cfg.gpb + q]
            zidx, eidx, dst3 = _pack_core(cfg, es, ed, lo, nc_b)
            poolw = np.zeros((cfg.NB, 128, 128), bf)
            for b in range(cfg.NB):
                for i in range(min(128, cfg.SH - b * 128)):
                    n = lo + b * 128 + i
                    if n < cfg.N:
                        poolw[b, i, gid[n]] = 1.0
            in_maps.append({
                "xfull": xfull,
                "waug1": waug1.astype(bf), "waugn": waugn.astype(bf),
                "wl3": wl3.astype(bf), "blp1": blp1.reshape(128, 1),
                "blpn": blpn.reshape(128, 1),
                "iota": iota, "ident": ident,
                "dst3": dst3, "zidx": zidx, "eidx": eidx, "poolw": poolw,
            })
    return nc_b, in_maps, host_meta


def _finalize(cfg, inputs, host_meta, pool_outs):
    """pool_outs: list of 8 [128,128] arrays -> full output [G,1] float64."""
    out = {}
    for br in range(2):
        sfx = "AB"[br]
        total = np.zeros((128, 128), np.float64)
        for q in range(cfg.gpb):
            total += pool_outs[br * cfg.gpb + q].astype(np.float64)
        gid = host_meta[sfx]["gid"]
        cnt = np.bincount(gid, minlength=128).astype(np.float64)
        total += cnt[:, None] * host_meta[sfx]["blpn"].astype(np.float64)[None, :]
        out[sfx] = (total / np.maximum(cnt[:, None], 1.0))[:cfg.G]
    cat = np.concatenate([out["A"], out["B"]], axis=1)
    Wo = np.asarray(inputs["Wo"]).astype(np.float64)
    bo = np.asarray(inputs["bo"]).astype(np.float64)
    return (cat @ Wo + bo).astype(np.float64)


_CACHE = {}


def kernel(**inputs):
    cfg = Cfg(N=inputs["featsA"].shape[0], G=128)
    nc_b, in_maps, host_meta = _prepare(cfg, inputs)
    key = ("prog", tuple(nc_b.tolist()))
    if key not in _CACHE:
        _CACHE[key] = build_program(cfg, nc_b)
    nc = _CACHE[key]
    from concourse.bass_utils import run_bass_kernel_spmd
    res = run_bass_kernel_spmd(nc, in_maps, list(range(cfg.n_cores)))
    pool_outs = [r["pool_out"] for r in res.results]
    return _finalize(cfg, inputs, host_meta, pool_outs)


# revision 85
# speedup vs baseline: 1.0029x; 1.0017x over previous
"""Trainium2 Bass kernel for nn_DoubleNet (two GATNet branches + avg-pool + linear).

Strategy (8 NeuronCores):
  - Cores 0-3 run branch A, cores 4-7 run branch B (same SPMD program,
    different input data per core). Within a branch, dst nodes are sharded
    contiguously across the 4 cores.
  - Per GAT layer:
      dense phase: stream x^T (bf16) from the all-gather piece tensors,
        compute z_aug = x @ [W|W@al|W@ar] on the PE (bf16 weights), and stage
        each 128-node chunk into a 512-byte table row: [z0|1|z1|1|z2|1] as
        fp8 in bytes 0:387, el/er as bf16 in bytes 388:400. Two DRAM tables
        ping-pong across layers so the next layer's dense phase can overlap
        the current edge phase.
      edge phase: edges are pre-sorted by dst (host side) and processed in
        chunks of 128, gathered 8 chunks per dma_gather call (the 1024-entry
        SWDGE ring bounds the call size): the full 512B row of each src
        (fp8 z + bf16 el), and the 256B second half-row of each dst (er).
        Gathers issue GLA groups ahead and the attention-weight chain
        (add + leaky_relu on DVE, exp on Act) WLA groups ahead of
        consumption. Per chunk, one of three engine-balanced aggregation
        styles ("o": 3 fused scaled one-hots on DVE + 3 PE matmuls against
        the raw fp8 rows; "v": unscaled one-hot + one broadcast
        tensor_tensor scale on DVE + 1 matmul; "a": one-hot on Pool + 3
        scaled copies on Act + 1 matmul) accumulates messages and softmax
        denominators (the interleaved ones columns) into a per-dst-block
        PSUM tile.
      block tail: denominator reciprocal (DVE), normalize to bf16 (Act),
        PE-transpose, evacuate (DVE), x_next^T = Wl^T @ agg^T (PE), bias
        (Act). Per-core x^T shards all-gather in three pieces; each piece
        triggers the next layer's dense work for the node chunks it covers,
        interleaved into the remaining edge-phase emission.
  - Final layer pools via a host-precomputed gid one-hot matmul; host sums the
    per-core partial pools and applies the output linear (float64).
"""

import sys

sys.path.insert(0, "/opt/trn_rl_repo")

import numpy as np


# ---------------------------------------------------------------------------
# configuration
# ---------------------------------------------------------------------------

class Cfg:
    def __init__(self, N=20000, G=128, H=3, EMB=128, F=128, n_cores=8,
                 neg_slope=0.2, GC=8, BST=4, TB=8, scale_eng="bal"):
        assert F == 128 and EMB == 128 and H == 3
        self.N, self.G, self.H, self.EMB, self.F = N, G, H, EMB, F
        self.n_cores = n_cores
        self.gpb = n_cores // 2            # cores per branch
        assert N % self.gpb == 0
        self.SH = N // self.gpb            # dst nodes per core
        self.NB = -(-self.SH // 128)       # dst blocks per core
        self.NT = -(-N // 128)             # node chunks for dense phase
        self.NTP = self.NT * 128           # padded node count
        self.neg_slope = neg_slope
        self.GC = GC                       # chunks per z-gather call
        self.BST = BST                     # chunks staged per table write
        self.TB = TB                       # chunks per dense x load
        self.scale_eng = scale_eng         # engines for per-head w-scaling
        self.ROW = 512                     # table row bytes (fp8 elements)
        self.dma_scratch = 16384           # SWDGE ring carveout bytes/partition
        self.psz_bufs = 3
        self.psb_bufs = 2
        self.pst_bufs = 1
        self.psx_bufs = 1
        self.GLA = 4                       # gather lookahead (groups)
        self.WLA = 2                       # attention-weight chain lookahead
        self.pat = "ooavooaoooavooaoo"
        self.pat2 = None                   # layer-2 override (no overlap work)
        self.pull = 5                      # dense chunks interleaved per group
        self.cuts = (22, 31)               # all-gather piece block boundaries
        self.zst_bufs = 12
        self.xin_bufs = 4
        self.g_bufs = 9
        self.gs_bufs = 16
        self.stg_split = False
        self.lh_split = False
        self.u_bufs = 16


# ---------------------------------------------------------------------------
# host-side data prep
# ---------------------------------------------------------------------------

def _prep_edges(cfg, src, dst, q):
    """Edges of one core (dst in its shard), dst-sorted, fake rows added."""
    lo = q * cfg.SH
    sel = (dst >= lo) & (dst < lo + cfg.SH)
    es = src[sel].astype(np.int64)
    ed = (dst[sel].astype(np.int64) - lo)
    nfake = cfg.NB * 128 - cfg.SH
    if nfake:
        es = np.concatenate([es, np.zeros(nfake, np.int64)])
        ed = np.concatenate([ed, np.arange(cfg.SH, cfg.NB * 128, dtype=np.int64)])
    order = np.argsort(ed, kind="stable")
    es, ed = es[order], ed[order]
    cnt = np.bincount(ed // 128, minlength=cfg.NB)
    return es, ed, cnt, lo


def _pack_core(cfg, es, ed, lo, nc_b):
    """Build flat (block, chunk, slot) arrays padded to nc_b chunks/block."""
    TOT = int(nc_b.sum())
    zsrc = np.zeros(TOT * 128, np.int16)
    edst = np.zeros(TOT * 128, np.int16)
    dst3 = np.full(TOT * 128, -1.0, np.float32)
    epos = np.searchsorted(ed, np.arange(0, cfg.NB * 128 + 1, 128))
    cum = np.concatenate([[0], np.cumsum(nc_b)]).astype(int)
    for b in range(cfg.NB):
        s, e = epos[b], epos[b + 1]
        n = e - s
        o = int(cum[b]) * 128
        zsrc[o:o + n] = es[s:e]
        # fake rows (local id >= SH) must not use an out-of-range er index
        ei = ed[s:e] + lo
        ei[ed[s:e] >= cfg.SH] = 0
        edst[o:o + n] = ei
        dst3[o:o + n] = (ed[s:e] - b * 128).astype(np.float32)
    # index tiles: flat i -> (partition i%16, col i//16), replicated to 128 rows
    def wrap(a):
        return np.tile(a.reshape(-1, 16).T, (8, 1)).copy()
    # dst3 partition-major: [128, TOT]
    d3 = dst3.reshape(TOT, 128).T.copy()
    return wrap(zsrc), wrap(edst), d3


def _prep_branch_weights(cfg, W1, al1, ar1, b1, Wn, aln, arn, bn, Wl, bl):
    H, EMB = cfg.H, cfg.EMB

    def waug(W, al, ar):
        K = W.shape[0]
        out = np.zeros((K, 390), np.float32)
        out[:, :384] = W
        for h in range(H):
            out[:, 384 + h] = W[:, h * EMB:(h + 1) * EMB] @ al[h]
            out[:, 387 + h] = W[:, h * EMB:(h + 1) * EMB] @ ar[h]
        return out

    wl3 = Wl.reshape(3, 128, EMB).astype(np.float32)
    blp1 = (b1 @ Wl + bl).astype(np.float32)
    blpn = (bn @ Wl + bl).astype(np.float32)
    return waug(W1, al1, ar1), waug(Wn, aln, arn), wl3, blp1, blpn


# ---------------------------------------------------------------------------
# device program
# ---------------------------------------------------------------------------

def build_program(cfg, nc_b, timing_mode=False):
    import concourse.bass as bass
    import concourse.mybir as mybir
    import concourse.tile as tile
    from concourse import bacc

    dt = mybir.dt
    f32 = dt.float32
    bf16 = dt.bfloat16
    f8 = dt.float8e4
    Alu = mybir.AluOpType
    Act = mybir.ActivationFunctionType

    NB, NT, SH, GC, BST, TB = cfg.NB, cfg.NT, cfg.SH, cfg.GC, cfg.BST, cfg.TB
    ROW = cfg.ROW
    TOT = int(nc_b.sum())
    cum = np.concatenate([[0], np.cumsum(nc_b)]).astype(int)
    # block index of each chunk
    blk_of = np.zeros(TOT, np.int64)
    for b in range(NB):
        blk_of[cum[b]:cum[b + 1]] = b
    gpb = cfg.gpb
    groups = [list(range(gpb)), list(range(gpb, 2 * gpb))]

    nc = bacc.Bacc("TRN2", target_bir_lowering=False, debug=False,
                   num_devices=cfg.n_cores,
                   dynamic_dma_scratch_size=cfg.dma_scratch)

    # inputs -----------------------------------------------------------------
    xfull = nc.dram_tensor("xfull", [128, cfg.NTP], bf16, kind="ExternalInput")
    waug1_d = nc.dram_tensor("waug1", [128, 390], bf16, kind="ExternalInput")
    waugn_d = nc.dram_tensor("waugn", [128, 390], bf16, kind="ExternalInput")
    wl3_d = nc.dram_tensor("wl3", [3, 128, 128], bf16, kind="ExternalInput")
    blp1_d = nc.dram_tensor("blp1", [128, 1], f32, kind="ExternalInput")
    blpn_d = nc.dram_tensor("blpn", [128, 1], f32, kind="ExternalInput")
    iota_d = nc.dram_tensor("iota", [128, 128], bf16, kind="ExternalInput")
    ident_d = nc.dram_tensor("ident", [128, 128], bf16, kind="ExternalInput")
    dst3_d = nc.dram_tensor("dst3", [128, TOT], f32, kind="ExternalInput")
    zidx_d = nc.dram_tensor("zidx", [128, TOT * 8], dt.int16, kind="ExternalInput")
    eidx_d = nc.dram_tensor("eidx", [128, TOT * 8], dt.int16, kind="ExternalInput")
    poolw_d = nc.dram_tensor("poolw", [NB, 128, 128], bf16, kind="ExternalInput")
    pool_out = nc.dram_tensor("pool_out", [128, 128], f32, kind="ExternalOutput")

    # internal DRAM ----------------------------------------------------------
    # Two z-tables ping-pong so layer L+1's dense phase (for the SH1 node
    # regions that all-gather mid-edge-phase) can overlap layer L's edge
    # phase without clobbering rows its gathers still read.
    zaug2 = [nc.dram_tensor("zaugA", [cfg.NTP, ROW], f8),
             nc.dram_tensor("zaugB", [cfg.NTP, ROW], f8)]
    # The per-core x^T shard all-gathers in P pieces so the next layer's
    # dense phase can start on each piece as soon as it lands.
    bcuts = [0, *cfg.cuts, NB]
    P = len(bcuts) - 1
    pieces = []                            # (b0, b1, col0, ncols)
    for p in range(P):
        b0, b1 = bcuts[p], bcuts[p + 1]
        col0 = b0 * 128
        ncols = min(b1 * 128, SH) - col0
        pieces.append((b0, b1, col0, ncols))
    xshp = [nc.dram_tensor(f"xsh{p}", [128, pieces[p][3]], bf16)
            for p in range(P)]
    xgathp = [nc.dram_tensor(f"xgath{p}", [gpb, 128, pieces[p][3]], bf16)
              for p in range(P)]

    def do_gather(nc, p):
        xs, xg = xshp[p], xgathp[p]
        if timing_mode:
            for j in range(gpb):
                nc.sync.dma_start(xg.ap()[j], xs.ap())
        else:
            nc.gpsimd.collective_compute(
                "AllGather", mybir.AluOpType.bypass, replica_groups=groups,
                ins=[xs.ap()], outs=[xg.ap()])

    # node-interval -> all-gather piece map (for dense-phase x loads)
    xpieces = []
    for j in range(gpb):
        for p in range(P):
            glo = j * SH + pieces[p][2]
            xpieces.append((glo, glo + pieces[p][3], xgathp[p], j))

    piece_of_block = np.zeros(NB, np.int64)
    for p in range(P):
        piece_of_block[bcuts[p]:bcuts[p + 1]] = p

    with tile.TileContext(nc) as tc:
        cpool = tc.alloc_tile_pool(name="const", bufs=1)
        waug1 = cpool.tile([128, 390], bf16, tag="waug1")
        waugn = cpool.tile([128, 390], bf16, tag="waugn")
        wl3 = cpool.tile([128, 3, 128], bf16, tag="wl3")
        blp1 = cpool.tile([128, 1], f32, tag="blp1")
        iota = cpool.tile([128, 128], bf16, tag="iota")
        ident = cpool.tile([128, 128], bf16, tag="ident")
        dst3 = cpool.tile([128, TOT], f32, tag="dst3")
        zidx = cpool.tile([128, TOT * 8], dt.int16, tag="zidx")
        eidx = cpool.tile([128, TOT * 8], dt.int16, tag="eidx")

        nc.sync.dma_start(waug1[:], waug1_d.ap())
        nc.sync.dma_start(waugn[:], waugn_d.ap())

        xin_pool = tc.alloc_tile_pool(name="xin", bufs=cfg.xin_bufs)
        psz_pool = tc.alloc_tile_pool(name="psz", bufs=cfg.psz_bufs, space="PSUM")
        zst_pool = tc.alloc_tile_pool(name="zst", bufs=cfg.zst_bufs)
        g_pool = tc.alloc_tile_pool(name="g", bufs=cfg.g_bufs)
        r_pool = tc.alloc_tile_pool(name="r", bufs=cfg.g_bufs)
        w_pool = tc.alloc_tile_pool(name="w", bufs=6)
        u_pool = tc.alloc_tile_pool(name="u", bufs=cfg.u_bufs)
        gs_pool = tc.alloc_tile_pool(name="gs", bufs=cfg.gs_bufs)
        psb_pool = tc.alloc_tile_pool(name="psb", bufs=cfg.psb_bufs, space="PSUM")
        s_pool = tc.alloc_tile_pool(name="s", bufs=2)
        a_pool = tc.alloc_tile_pool(name="a", bufs=2)
        if cfg.pst_bufs:
            pst_pool = tc.alloc_tile_pool(name="pst", bufs=cfg.pst_bufs,
                                          space="PSUM")
        at_pool = tc.alloc_tile_pool(name="at", bufs=2)
        psx_pool = tc.alloc_tile_pool(name="psx", bufs=cfg.psx_bufs,
                                      space="PSUM")
        if not cfg.pst_bufs:
            pst_pool = psx_pool        # transposes share the psx banks
        x_pool = tc.alloc_tile_pool(name="x", bufs=2)
        pw_pool = tc.alloc_tile_pool(name="pw", bufs=2)
        pp_pool = tc.alloc_tile_pool(name="pp", bufs=1, space="PSUM")

        ps_pool_acc = pp_pool.tile([128, 128], f32, tag="poolacc")

        # Per-chunk aggregation styles, rotated to balance engines:
        #   "o": 3 fused scaled-one-hots on DVE (is_equal*w) + 3 PE matmuls
        #        against the raw fp8 rows -- cheap on DVE, heavy on PE SEQ.
        #   "v": unscaled one-hot + ONE fused broadcast tensor_tensor scale
        #        on DVE + 1 PE matmul.
        #   "a": unscaled one-hot (Pool) + 3 scaled copies on Act + 1 matmul.
        # Pool otherwise only generates gather descriptors (its in-order
        # sequencer must not block behind data-dependent work).
        _PAT = cfg.pat

        def scale_chunk(eng, c, cl, Gs, Gt, wt):
            if eng == "a":
                for h in range(3):
                    sl = slice(129 * h, 129 * h + 129)
                    nc.scalar.activation(Gs[:, sl], Gt[:, cl, sl].opt(),
                                         Act.Copy,
                                         scale=wt[:, cl, h:h + 1].opt())
            else:
                dst = Gs[:, 0:387].rearrange("p (h c) -> p h c", h=3)
                src = Gt[:, cl, 0:387].rearrange("p (h c) -> p h c", h=3)
                wb = wt[:, cl, :].unsqueeze(-1).broadcast_to([128, 3, 129])
                if eng == "v":
                    nc.vector.tensor_tensor(dst, src, wb, Alu.mult)
                else:
                    nc.gpsimd.tensor_tensor(dst, src, wb, Alu.mult)

        # Dense-phase chunk runs per all-gather piece: a node chunk becomes
        # computable once every piece covering it has landed; chunks fully
        # inside one piece's region go to that piece, stragglers go last.
        assigned = np.full(NT, P - 1, np.int64)
        for p in range(P):
            for j in range(gpb):
                glo = j * SH + pieces[p][2]
                ghi = glo + pieces[p][3]
                lo = -(-glo // 128)
                hi = ghi // 128
                assigned[lo:hi] = p

        def runs_of(p):
            runs, s = [], None
            for t in range(NT):
                if assigned[t] == p and s is None:
                    s = t
                elif assigned[t] != p and s is not None:
                    runs.append((s, t))
                    s = None
            if s is not None:
                runs.append((s, NT))
            return runs

        piece_runs = [runs_of(p) for p in range(P)]

        def load_x(layer, xin, t, tb):
            """Fill xin[:, 0:tb, :] with x^T nodes [t*128, (t+tb)*128),
            reading xfull (layer 0) or the all-gather piece tensors."""
            a, bnd = t * 128, (t + tb) * 128
            flat = xin[:, 0:tb, :].rearrange("p c n -> p (c n)")
            if layer == 0:
                nc.sync.dma_start(flat, xfull.ap()[:, a:bnd])
                return
            for glo, ghi, tens, j in xpieces:
                lo, hi = max(a, glo), min(bnd, ghi)
                if lo < hi:
                    nc.sync.dma_start(flat[:, lo - a:hi - a],
                                      tens.ap()[j][:, lo - glo:hi - glo])

        def dense_gen(layer, runs, in_edge=False):
            """Generator emitting the fp8 z_aug table build for `runs` of
            node chunks; yields after each staged chunk. in_edge: emitted
            interleaved with the edge phase, where Pool is saturated with
            gather descriptor generation -- keep memsets off Pool there."""
            wa = waug1 if layer == 0 else waugn
            tab = zaug2[layer % 2]
            for r0_, r1_ in runs:
                t = r0_
                while t < r1_:
                    tb = min(TB, r1_ - t)
                    xin = xin_pool.tile([128, TB, 128], bf16, tag="xin")
                    load_x(layer, xin, t, tb)
                    s0 = 0
                    while s0 < tb:
                        sb = min(BST, tb - s0)
                        zt = zst_pool.tile([128, BST, ROW], f8, tag="zt")
                        ztb = zt.bitcast(bf16)
                        for s in range(sb):
                            psz = psz_pool.tile([128, 390], f32, tag="psz")
                            nc.tensor.matmul(psz[:], xin[:, s0 + s, :].opt(),
                                             wa[:], start=True, stop=True)
                            nc.gpsimd.memset(zt[:, s, 128:387:129], 1.0)
                            if cfg.stg_split and not in_edge:
                                # both engines stage each chunk (shorter psz
                                # hold): Act heads 0-1, DVE head 2 + el/er
                                nc.scalar.activation(
                                    zt[:, s, 0:258]
                                    .rearrange("p (g c) -> p g c", g=2)
                                    [:, :, 0:128],
                                    psz[:, 0:256]
                                    .rearrange("p (g c) -> p g c", g=2),
                                    Act.Copy)
                                nc.vector.tensor_copy(zt[:, s, 258:386],
                                                      psz[:, 256:384])
                                nc.vector.tensor_copy(ztb[:, s, 194:200],
                                                      psz[:, 384:390])
                                continue_yield = None
                            else:
                                zdst = (zt[:, s, 0:387]
                                        .rearrange("p (g c) -> p g c", g=3)
                                        [:, :, 0:128])
                                zsrc = (psz[:, 0:384]
                                        .rearrange("p (g c) -> p g c", g=3))
                                if (t + s0 + s) % 2:
                                    nc.scalar.activation(zdst, zsrc, Act.Copy)
                                    nc.vector.tensor_copy(ztb[:, s, 194:200],
                                                          psz[:, 384:390])
                                else:
                                    nc.vector.tensor_copy(zdst, zsrc)
                                    nc.scalar.activation(ztb[:, s, 194:200],
                                                         psz[:, 384:390],
                                                         Act.Copy)
                            yield
                        r0 = (t + s0) * 128
                        nc.sync.dma_start(
                            tab.ap()[r0:r0 + sb * 128, :]
                            .rearrange("(c p) z -> p c z", p=128),
                            zt[:, 0:sb, :])
                        s0 += sb
                    t += tb

        for _ in dense_gen(0, [(0, NT)]):
            pass

        # edge-phase-only constants load during the layer-0 dense phase so
        # they don't delay its first x loads on the SP queue
        nc.sync.dma_start(wl3[:], wl3_d.ap().rearrange("k p m -> p k m"))
        nc.sync.dma_start(blp1[:], blp1_d.ap())
        nc.sync.dma_start(iota[:], iota_d.ap())
        nc.sync.dma_start(ident[:], ident_d.ap())
        nc.sync.dma_start(dst3[:], dst3_d.ap())
        nc.sync.dma_start(zidx[:], zidx_d.ap())
        nc.sync.dma_start(eidx[:], eidx_d.ap())

        for layer in range(3):
            zaug = zaug2[layer % 2]
            # next layer's dense work, one generator per landed gather piece,
            # interleaved into this layer's edge phase
            if layer < 2:
                ogens = [[bcuts[p + 1] + 1,
                          dense_gen(layer + 1, piece_runs[p], in_edge=True)]
                         for p in range(P - 1)]
            else:
                ogens = []
            tail_done = [-1]

            # ---------------- edge phase ------------------------------------
            # Software pipeline: gathers issue GLA groups ahead and the
            # attention-weight chain (add/lrelu/exp) WLA groups ahead of the
            # group whose chunks are being consumed, so in-order engine
            # streams never sit at a semaphore for data that could have been
            # requested earlier.
            n_groups = -(-TOT // GC)
            GLA, WLA = cfg.GLA, cfg.WLA
            inflight = {}

            def issue_gathers(g):
                c0 = g * GC
                gsz = min(GC, TOT - c0)
                Gt = g_pool.tile([128, GC, ROW], f8, tag="G")
                R = r_pool.tile([128, GC, 256], f8, tag="R")
                nc.gpsimd.dma_gather(
                    Gt[:, 0:gsz, :], zaug.ap(),
                    zidx[:, 8 * c0: 8 * (c0 + gsz)],
                    num_idxs=gsz * 128, num_idxs_reg=gsz * 128,
                    elem_size=ROW, elem_step=ROW)
                nc.gpsimd.dma_gather(
                    R[:, 0:gsz, :], zaug.ap()[:, 256:512],
                    eidx[:, 8 * c0: 8 * (c0 + gsz)],
                    num_idxs=gsz * 128, num_idxs_reg=gsz * 128,
                    elem_size=256, elem_step=ROW)
                inflight[g] = [Gt, R, None]

            def issue_wt(g):
                c0 = g * GC
                gsz = min(GC, TOT - c0)
                Gt, R, _ = inflight[g]
                Gtb = Gt.bitcast(bf16)       # [128, GC, 256]
                Rb = R.bitcast(bf16)         # [128, GC, 128]
                wt = w_pool.tile([128, GC, 3], f32, tag="wt")
                nc.vector.tensor_tensor(
                    wt[:, 0:gsz, :], Gtb[:, 0:gsz, 194:197],
                    Rb[:, 0:gsz, 69:72], Alu.add)
                nc.vector.scalar_tensor_tensor(
                    wt[:, 0:gsz, :], wt[:, 0:gsz, :], cfg.neg_slope,
                    wt[:, 0:gsz, :], Alu.mult, Alu.max)
                nc.scalar.activation(wt[:, 0:gsz, :], wt[:, 0:gsz, :], Act.Exp)
                inflight[g][2] = wt

            for g in range(min(GLA, n_groups)):
                issue_gathers(g)
            for g in range(min(WLA, n_groups)):
                issue_wt(g)
            psb = None
            for gi in range(n_groups):
                if gi + GLA < n_groups:
                    issue_gathers(gi + GLA)
                if gi + WLA < n_groups:
                    issue_wt(gi + WLA)
                pulled = 0
                for og in ogens:
                    if og[1] is None or tail_done[0] < og[0]:
                        continue
                    while pulled < cfg.pull:
                        if next(og[1], "done") == "done":
                            og[1] = None
                            break
                        pulled += 1
                    if pulled >= cfg.pull:
                        break
                c0 = gi * GC
                gsz = min(GC, TOT - c0)
                Gt, R, wt = inflight.pop(gi)
                for cl in range(gsz):
                    c = c0 + cl
                    b = int(blk_of[c])
                    first = c == int(cum[b])
                    last = c == int(cum[b + 1]) - 1
                    if first:
                        psb = psb_pool.tile([128, 387], f32, tag="psb")
                    eng = cfg.scale_eng
                    if eng == "bal":
                        lp = (cfg.pat2 if layer == 2 and cfg.pat2 else _PAT)
                        eng = lp[c % len(lp)]
                    if eng == "o":
                        if cfg.lh_split:
                            lhs = []
                            for h in range(3):
                                lht = u_pool.tile([128, 128], bf16,
                                                  tag="u", name=f"lh{h}")
                                lhs.append(lht)
                        else:
                            lh3 = gs_pool.tile([128, 3, 128], bf16, tag="Gs")
                            lhs = [lh3[:, h, :].opt() for h in range(3)]
                        for h in range(3):
                            lh = lhs[h][:] if cfg.lh_split else lhs[h]
                            nc.vector.tensor_scalar(
                                lh, iota[:], dst3[:, c:c + 1],
                                wt[:, cl, h:h + 1].opt(),
                                Alu.is_equal, Alu.mult)
                        for h in range(3):
                            sl = slice(129 * h, 129 * h + 129)
                            lh = lhs[h][:] if cfg.lh_split else lhs[h]
                            nc.tensor.matmul(psb[:, sl], lh,
                                             Gt[:, cl, sl].opt(),
                                             start=(first and h == 0),
                                             stop=(last and h == 2))
                    else:
                        u = u_pool.tile([128, 128], bf16, tag="u")
                        if eng == "a":
                            nc.gpsimd.tensor_scalar(u[:], iota[:],
                                                    dst3[:, c:c + 1],
                                                    None, Alu.is_equal)
                        else:
                            nc.vector.tensor_scalar(u[:], iota[:],
                                                    dst3[:, c:c + 1],
                                                    None, Alu.is_equal)
                        Gs = gs_pool.tile([128, 390], bf16, tag="Gs")
                        scale_chunk(eng, c, cl, Gs, Gt, wt)
                        nc.tensor.matmul(psb[:, 0:387], u[:], Gs[:, 0:387],
                                         start=first, stop=last)
                    if last:
                        # ---------------- block tail ------------------------
                        r3 = s_pool.tile([128, 3], f32, tag="r3")
                        nc.vector.reciprocal(r3[:], psb[:, 128:387:129])
                        agg = a_pool.tile([128, 3, 128], bf16, tag="agg")
                        for h in range(3):
                            nc.scalar.activation(
                                agg[:, h, :].opt(),
                                psb[:, 129 * h:129 * h + 128],
                                Act.Copy, scale=r3[:, h:h + 1].opt())
                        aggT = at_pool.tile([128, 3, 128], bf16, tag="aggT")
                        for h in range(3):
                            pst = pst_pool.tile([128, 128], bf16, tag="pst")
                            nc.tensor.transpose(pst[:], agg[:, h, :].opt(),
                                                ident[:])
                            nc.vector.tensor_copy(aggT[:, h, :].opt(), pst[:])
                        bw = min(128, SH - b * 128)
                        psx = psx_pool.tile([128, 128], f32, tag="psx")
                        if layer < 2:
                            for k in range(3):
                                nc.tensor.matmul(psx[:], wl3[:, k, :].opt(),
                                                 aggT[:, k, :].opt(),
                                                 start=(k == 0), stop=(k == 2))
                            xsb = x_pool.tile([128, 128], bf16, tag="xsb")
                            nc.scalar.activation(xsb[:], psx[:], Act.Identity,
                                                 bias=blp1[:])
                            p = int(piece_of_block[b])
                            o = b * 128 - pieces[p][2]
                            nc.sync.dma_start(
                                xshp[p].ap()[:, o:o + bw], xsb[:, 0:bw])
                            if p < P - 1 and b == bcuts[p + 1] - 1:
                                do_gather(nc, p)
                            tail_done[0] = b
                        else:
                            for k in range(3):
                                nc.tensor.matmul(psx[:], aggT[:, k, :].opt(),
                                                 wl3[:, k, :].opt(),
                                                 start=(k == 0), stop=(k == 2))
                            x3 = x_pool.tile([128, 128], bf16, tag="xsb")
                            nc.vector.tensor_copy(x3[:], psx[:])
                            pw = pw_pool.tile([128, 128], bf16, tag="pw")
                            nc.sync.dma_start(pw[:], poolw_d.ap()[b])
                            nc.tensor.matmul(ps_pool_acc[:], pw[:], x3[:],
                                             start=(b == 0), stop=(b == NB - 1))

            for og in ogens:
                if og[1] is not None:
                    for _ in og[1]:
                        pass
            if layer < 2:
                do_gather(nc, P - 1)
                if layer == 0:
                    nc.sync.dma_start(blp1[:], blpn_d.ap())
                for _ in dense_gen(layer + 1, piece_runs[P - 1]):
                    pass

        po = x_pool.tile([128, 128], f32, tag="po")
        nc.vector.tensor_copy(po[:], ps_pool_acc[:])
        nc.sync.dma_start(pool_out.ap(), po[:])

        _rel_pst = (pst_pool,) if cfg.pst_bufs else ()
        for p in (pp_pool, pw_pool, x_pool, psx_pool, at_pool, *_rel_pst,
                  a_pool, s_pool, psb_pool, gs_pool, u_pool, w_pool,
                  r_pool, g_pool, zst_pool, psz_pool, xin_pool, cpool):
            p.release()

    nc.compile()
    return nc


# ---------------------------------------------------------------------------
# top-level kernel
# ---------------------------------------------------------------------------

def _prepare(cfg, inputs):
    """Returns (nc_b, in_maps, host_meta)."""
    import ml_dtypes
    bf = ml_dtypes.bfloat16
    npf = np.asarray
    per_core_edges = []
    nc_b = np.zeros(cfg.NB, np.int64)
    for br, (s, d) in enumerate((("srcA", "dstA"), ("srcB", "dstB"))):
        src = npf(inputs[s]).astype(np.int64)
        dst = npf(inputs[d]).astype(np.int64)
        for q in range(cfg.gpb):
            es, ed, cnt, lo = _prep_edges(cfg, src, dst, q)
            per_core_edges.append((es, ed, lo))
            nc_b = np.maximum(nc_b, -(-cnt // 128))
    in_maps = []
    host_meta = {}
    iota = np.tile(np.arange(128, dtype=bf), (128, 1))
    ident = np.eye(128, dtype=bf)
    for br in range(2):
        sfx = "AB"[br]
        W1 = npf(inputs["W1" + sfx]); al1 = npf(inputs["al1" + sfx])
        ar1 = npf(inputs["ar1" + sfx]); b1 = npf(inputs["b1" + sfx])
        Wn = npf(inputs["Wn" + sfx]); aln = npf(inputs["aln" + sfx])
        arn = npf(inputs["arn" + sfx]); bn = npf(inputs["bn" + sfx])
        Wl = npf(inputs["Wl" + sfx]); bl = npf(inputs["bl" + sfx])
        gid = npf(inputs["gid" + sfx]).astype(np.int64)
        feats = npf(inputs["feats" + sfx]).astype(np.float32)
        waug1, waugn, wl3, blp1, blpn = _prep_branch_weights(
            cfg, W1, al1, ar1, b1, Wn, aln, arn, bn, Wl, bl)
        xfull = np.zeros((128, cfg.NTP), bf)
        xfull[:, :cfg.N] = feats.T.astype(bf)
        host_meta[sfx] = dict(blpn=blpn, gid=gid)
        for q in range(cfg.gpb):
            es, ed, lo = per_core_edges[br * cfg.gpb + q]
            zidx, eidx, dst3 = _pack_core(cfg, es, ed, lo, nc_b)
            poolw = np.zeros((cfg.NB, 128, 128), bf)
            for b in range(cfg.NB):
                for i in range(min(128, cfg.SH - b * 128)):
                    n = lo + b * 128 + i
                    if n < cfg.N:
                        poolw[b, i, gid[n]] = 1.0
            in_maps.append({
                "xfull": xfull,
                "waug1": waug1.astype(bf), "waugn": waugn.astype(bf),
                "wl3": wl3.astype(bf), "blp1": blp1.reshape(128, 1),
                "blpn": blpn.reshape(128, 1),
                "iota": iota, "ident": ident,
                "dst3": dst3, "zidx": zidx, "eidx": eidx, "poolw": poolw,
            })
    return nc_b, in_maps, host_meta


def _finalize(cfg, inputs, host_meta, pool_outs):
    """pool_outs: list of 8 [128,128] arrays -> full output [G,1] float64."""
    out = {}
    for br in range(2):
        sfx = "AB"[br]
        total = np.zeros((128, 128), np.float64)
        for q in range(cfg.gpb):
            total += pool_outs[br * cfg.gpb + q].astype(np.float64)
        gid = host_meta[sfx]["gid"]
        cnt = np.bincount(gid, minlength=128).astype(np.float64)
        total += cnt[:, None] * host_meta[sfx]["blpn"].astype(np.float64)[None, :]
        out[sfx] = (total / np.maximum(cnt[:, None], 1.0))[:cfg.G]
    cat = np.concatenate([out["A"], out["B"]], axis=1)
    Wo = np.asarray(inputs["Wo"]).astype(np.float64)
    bo = np.asarray(inputs["bo"]).astype(np.float64)
    return (cat @ Wo + bo).astype(np.float64)


_CACHE = {}


def kernel(**inputs):
    cfg = Cfg(N=inputs["featsA"].shape[0], G=128)
    nc_b, in_maps, host_meta = _prepare(cfg, inputs)
    key = ("prog", tuple(nc_b.tolist()))
    if key not in _CACHE:
        _CACHE[key] = build_program(cfg, nc_b)
    nc = _CACHE[key]
    from concourse.bass_utils import run_bass_kernel_spmd
    res = run_bass_kernel_spmd(nc, in_maps, list(range(cfg.n_cores)))
    pool_outs = [r["pool_out"] for r in res.results]
    return _finalize(cfg, inputs, host_meta, pool_outs)


# revision 86
# speedup vs baseline: 1.0045x; 1.0016x over previous
"""Trainium2 Bass kernel for nn_DoubleNet (two GATNet branches + avg-pool + linear).

Strategy (8 NeuronCores):
  - Cores 0-3 run branch A, cores 4-7 run branch B (same SPMD program,
    different input data per core). Within a branch, dst nodes are sharded
    contiguously across the 4 cores.
  - Per GAT layer:
      dense phase: stream x^T (bf16) from the all-gather piece tensors,
        compute z_aug = x @ [W|W@al|W@ar] on the PE (bf16 weights), and stage
        each 128-node chunk into a 512-byte table row: [z0|1|z1|1|z2|1] as
        fp8 in bytes 0:387, el/er as bf16 in bytes 388:400. Two DRAM tables
        ping-pong across layers so the next layer's dense phase can overlap
        the current edge phase.
      edge phase: edges are pre-sorted by dst (host side) and processed in
        chunks of 128, gathered 8 chunks per dma_gather call (the 1024-entry
        SWDGE ring bounds the call size): the full 512B row of each src
        (fp8 z + bf16 el), and the 256B second half-row of each dst (er).
        Gathers issue GLA groups ahead and the attention-weight chain
        (add + leaky_relu on DVE, exp on Act) WLA groups ahead of
        consumption. Per chunk, one of three engine-balanced aggregation
        styles ("o": 3 fused scaled one-hots on DVE + 3 PE matmuls against
        the raw fp8 rows; "v": unscaled one-hot + one broadcast
        tensor_tensor scale on DVE + 1 matmul; "a": one-hot on Pool + 3
        scaled copies on Act + 1 matmul) accumulates messages and softmax
        denominators (the interleaved ones columns) into a per-dst-block
        PSUM tile.
      block tail: denominator reciprocal (DVE), normalize to bf16 (Act),
        PE-transpose, evacuate (DVE), x_next^T = Wl^T @ agg^T (PE), bias
        (Act). Per-core x^T shards all-gather in three pieces; each piece
        triggers the next layer's dense work for the node chunks it covers,
        interleaved into the remaining edge-phase emission.
  - Final layer pools via a host-precomputed gid one-hot matmul; host sums the
    per-core partial pools and applies the output linear (float64).
"""

import sys

sys.path.insert(0, "/opt/trn_rl_repo")

import numpy as np


# ---------------------------------------------------------------------------
# configuration
# ---------------------------------------------------------------------------

class Cfg:
    def __init__(self, N=20000, G=128, H=3, EMB=128, F=128, n_cores=8,
                 neg_slope=0.2, GC=8, BST=4, TB=8, scale_eng="bal"):
        assert F == 128 and EMB == 128 and H == 3
        self.N, self.G, self.H, self.EMB, self.F = N, G, H, EMB, F
        self.n_cores = n_cores
        self.gpb = n_cores // 2            # cores per branch
        assert N % self.gpb == 0
        self.SH = N // self.gpb            # dst nodes per core
        self.NB = -(-self.SH // 128)       # dst blocks per core
        self.NT = -(-N // 128)             # node chunks for dense phase
        self.NTP = self.NT * 128           # padded node count
        self.neg_slope = neg_slope
        self.GC = GC                       # chunks per z-gather call
        self.BST = BST                     # chunks staged per table write
        self.TB = TB                       # chunks per dense x load
        self.scale_eng = scale_eng         # engines for per-head w-scaling
        self.ROW = 512                     # table row bytes (fp8 elements)
        self.dma_scratch = 16384           # SWDGE ring carveout bytes/partition
        self.psz_bufs = 3
        self.psb_bufs = 2
        self.pst_bufs = 1
        self.psx_bufs = 1
        self.GLA = 4                       # gather lookahead (groups)
        self.WLA = 2                       # attention-weight chain lookahead
        self.pat = "ooavooaoooavooaoo"
        self.pat2 = None                   # layer-2 override (no overlap work)
        self.pull = 5                      # dense chunks interleaved per group
        self.cuts = (22, 31)               # all-gather piece block boundaries
        self.zst_bufs = 12
        self.xin_bufs = 4
        self.g_bufs = 9
        self.gs_bufs = 18
        self.stg_split = False
        self.lh_split = False
        self.u_bufs = 16


# ---------------------------------------------------------------------------
# host-side data prep
# ---------------------------------------------------------------------------

def _prep_edges(cfg, src, dst, q):
    """Edges of one core (dst in its shard), dst-sorted, fake rows added."""
    lo = q * cfg.SH
    sel = (dst >= lo) & (dst < lo + cfg.SH)
    es = src[sel].astype(np.int64)
    ed = (dst[sel].astype(np.int64) - lo)
    nfake = cfg.NB * 128 - cfg.SH
    if nfake:
        es = np.concatenate([es, np.zeros(nfake, np.int64)])
        ed = np.concatenate([ed, np.arange(cfg.SH, cfg.NB * 128, dtype=np.int64)])
    order = np.argsort(ed, kind="stable")
    es, ed = es[order], ed[order]
    cnt = np.bincount(ed // 128, minlength=cfg.NB)
    return es, ed, cnt, lo


def _pack_core(cfg, es, ed, lo, nc_b):
    """Build flat (block, chunk, slot) arrays padded to nc_b chunks/block."""
    TOT = int(nc_b.sum())
    zsrc = np.zeros(TOT * 128, np.int16)
    edst = np.zeros(TOT * 128, np.int16)
    dst3 = np.full(TOT * 128, -1.0, np.float32)
    epos = np.searchsorted(ed, np.arange(0, cfg.NB * 128 + 1, 128))
    cum = np.concatenate([[0], np.cumsum(nc_b)]).astype(int)
    for b in range(cfg.NB):
        s, e = epos[b], epos[b + 1]
        n = e - s
        o = int(cum[b]) * 128
        zsrc[o:o + n] = es[s:e]
        # fake rows (local id >= SH) must not use an out-of-range er index
        ei = ed[s:e] + lo
        ei[ed[s:e] >= cfg.SH] = 0
        edst[o:o + n] = ei
        dst3[o:o + n] = (ed[s:e] - b * 128).astype(np.float32)
    # index tiles: flat i -> (partition i%16, col i//16), replicated to 128 rows
    def wrap(a):
        return np.tile(a.reshape(-1, 16).T, (8, 1)).copy()
    # dst3 partition-major: [128, TOT]
    d3 = dst3.reshape(TOT, 128).T.copy()
    return wrap(zsrc), wrap(edst), d3


def _prep_branch_weights(cfg, W1, al1, ar1, b1, Wn, aln, arn, bn, Wl, bl):
    H, EMB = cfg.H, cfg.EMB

    def waug(W, al, ar):
        K = W.shape[0]
        out = np.zeros((K, 390), np.float32)
        out[:, :384] = W
        for h in range(H):
            out[:, 384 + h] = W[:, h * EMB:(h + 1) * EMB] @ al[h]
            out[:, 387 + h] = W[:, h * EMB:(h + 1) * EMB] @ ar[h]
        return out

    wl3 = Wl.reshape(3, 128, EMB).astype(np.float32)
    blp1 = (b1 @ Wl + bl).astype(np.float32)
    blpn = (bn @ Wl + bl).astype(np.float32)
    return waug(W1, al1, ar1), waug(Wn, aln, arn), wl3, blp1, blpn


# ---------------------------------------------------------------------------
# device program
# ---------------------------------------------------------------------------

def build_program(cfg, nc_b, timing_mode=False):
    import concourse.bass as bass
    import concourse.mybir as mybir
    import concourse.tile as tile
    from concourse import bacc

    dt = mybir.dt
    f32 = dt.float32
    bf16 = dt.bfloat16
    f8 = dt.float8e4
    Alu = mybir.AluOpType
    Act = mybir.ActivationFunctionType

    NB, NT, SH, GC, BST, TB = cfg.NB, cfg.NT, cfg.SH, cfg.GC, cfg.BST, cfg.TB
    ROW = cfg.ROW
    TOT = int(nc_b.sum())
    cum = np.concatenate([[0], np.cumsum(nc_b)]).astype(int)
    # block index of each chunk
    blk_of = np.zeros(TOT, np.int64)
    for b in range(NB):
        blk_of[cum[b]:cum[b + 1]] = b
    gpb = cfg.gpb
    groups = [list(range(gpb)), list(range(gpb, 2 * gpb))]

    nc = bacc.Bacc("TRN2", target_bir_lowering=False, debug=False,
                   num_devices=cfg.n_cores,
                   dynamic_dma_scratch_size=cfg.dma_scratch)

    # inputs -----------------------------------------------------------------
    xfull = nc.dram_tensor("xfull", [128, cfg.NTP], bf16, kind="ExternalInput")
    waug1_d = nc.dram_tensor("waug1", [128, 390], bf16, kind="ExternalInput")
    waugn_d = nc.dram_tensor("waugn", [128, 390], bf16, kind="ExternalInput")
    wl3_d = nc.dram_tensor("wl3", [3, 128, 128], bf16, kind="ExternalInput")
    blp1_d = nc.dram_tensor("blp1", [128, 1], f32, kind="ExternalInput")
    blpn_d = nc.dram_tensor("blpn", [128, 1], f32, kind="ExternalInput")
    iota_d = nc.dram_tensor("iota", [128, 128], bf16, kind="ExternalInput")
    ident_d = nc.dram_tensor("ident", [128, 128], bf16, kind="ExternalInput")
    dst3_d = nc.dram_tensor("dst3", [128, TOT], f32, kind="ExternalInput")
    zidx_d = nc.dram_tensor("zidx", [128, TOT * 8], dt.int16, kind="ExternalInput")
    eidx_d = nc.dram_tensor("eidx", [128, TOT * 8], dt.int16, kind="ExternalInput")
    poolw_d = nc.dram_tensor("poolw", [NB, 128, 128], bf16, kind="ExternalInput")
    pool_out = nc.dram_tensor("pool_out", [128, 128], f32, kind="ExternalOutput")

    # internal DRAM ----------------------------------------------------------
    # Two z-tables ping-pong so layer L+1's dense phase (for the SH1 node
    # regions that all-gather mid-edge-phase) can overlap layer L's edge
    # phase without clobbering rows its gathers still read.
    zaug2 = [nc.dram_tensor("zaugA", [cfg.NTP, ROW], f8),
             nc.dram_tensor("zaugB", [cfg.NTP, ROW], f8)]
    # The per-core x^T shard all-gathers in P pieces so the next layer's
    # dense phase can start on each piece as soon as it lands.
    bcuts = [0, *cfg.cuts, NB]
    P = len(bcuts) - 1
    pieces = []                            # (b0, b1, col0, ncols)
    for p in range(P):
        b0, b1 = bcuts[p], bcuts[p + 1]
        col0 = b0 * 128
        ncols = min(b1 * 128, SH) - col0
        pieces.append((b0, b1, col0, ncols))
    xshp = [nc.dram_tensor(f"xsh{p}", [128, pieces[p][3]], bf16)
            for p in range(P)]
    xgathp = [nc.dram_tensor(f"xgath{p}", [gpb, 128, pieces[p][3]], bf16)
              for p in range(P)]

    def do_gather(nc, p):
        xs, xg = xshp[p], xgathp[p]
        if timing_mode:
            for j in range(gpb):
                nc.sync.dma_start(xg.ap()[j], xs.ap())
        else:
            nc.gpsimd.collective_compute(
                "AllGather", mybir.AluOpType.bypass, replica_groups=groups,
                ins=[xs.ap()], outs=[xg.ap()])

    # node-interval -> all-gather piece map (for dense-phase x loads)
    xpieces = []
    for j in range(gpb):
        for p in range(P):
            glo = j * SH + pieces[p][2]
            xpieces.append((glo, glo + pieces[p][3], xgathp[p], j))

    piece_of_block = np.zeros(NB, np.int64)
    for p in range(P):
        piece_of_block[bcuts[p]:bcuts[p + 1]] = p

    with tile.TileContext(nc) as tc:
        cpool = tc.alloc_tile_pool(name="const", bufs=1)
        waug1 = cpool.tile([128, 390], bf16, tag="waug1")
        waugn = cpool.tile([128, 390], bf16, tag="waugn")
        wl3 = cpool.tile([128, 3, 128], bf16, tag="wl3")
        blp1 = cpool.tile([128, 1], f32, tag="blp1")
        iota = cpool.tile([128, 128], bf16, tag="iota")
        ident = cpool.tile([128, 128], bf16, tag="ident")
        dst3 = cpool.tile([128, TOT], f32, tag="dst3")
        zidx = cpool.tile([128, TOT * 8], dt.int16, tag="zidx")
        eidx = cpool.tile([128, TOT * 8], dt.int16, tag="eidx")

        nc.sync.dma_start(waug1[:], waug1_d.ap())
        nc.sync.dma_start(waugn[:], waugn_d.ap())

        xin_pool = tc.alloc_tile_pool(name="xin", bufs=cfg.xin_bufs)
        psz_pool = tc.alloc_tile_pool(name="psz", bufs=cfg.psz_bufs, space="PSUM")
        zst_pool = tc.alloc_tile_pool(name="zst", bufs=cfg.zst_bufs)
        g_pool = tc.alloc_tile_pool(name="g", bufs=cfg.g_bufs)
        r_pool = tc.alloc_tile_pool(name="r", bufs=cfg.g_bufs)
        w_pool = tc.alloc_tile_pool(name="w", bufs=6)
        u_pool = tc.alloc_tile_pool(name="u", bufs=cfg.u_bufs)
        gs_pool = tc.alloc_tile_pool(name="gs", bufs=cfg.gs_bufs)
        psb_pool = tc.alloc_tile_pool(name="psb", bufs=cfg.psb_bufs, space="PSUM")
        s_pool = tc.alloc_tile_pool(name="s", bufs=2)
        a_pool = tc.alloc_tile_pool(name="a", bufs=2)
        if cfg.pst_bufs:
            pst_pool = tc.alloc_tile_pool(name="pst", bufs=cfg.pst_bufs,
                                          space="PSUM")
        at_pool = tc.alloc_tile_pool(name="at", bufs=2)
        psx_pool = tc.alloc_tile_pool(name="psx", bufs=cfg.psx_bufs,
                                      space="PSUM")
        if not cfg.pst_bufs:
            pst_pool = psx_pool        # transposes share the psx banks
        x_pool = tc.alloc_tile_pool(name="x", bufs=2)
        pw_pool = tc.alloc_tile_pool(name="pw", bufs=2)
        pp_pool = tc.alloc_tile_pool(name="pp", bufs=1, space="PSUM")

        ps_pool_acc = pp_pool.tile([128, 128], f32, tag="poolacc")

        # Per-chunk aggregation styles, rotated to balance engines:
        #   "o": 3 fused scaled-one-hots on DVE (is_equal*w) + 3 PE matmuls
        #        against the raw fp8 rows -- cheap on DVE, heavy on PE SEQ.
        #   "v": unscaled one-hot + ONE fused broadcast tensor_tensor scale
        #        on DVE + 1 PE matmul.
        #   "a": unscaled one-hot (Pool) + 3 scaled copies on Act + 1 matmul.
        # Pool otherwise only generates gather descriptors (its in-order
        # sequencer must not block behind data-dependent work).
        _PAT = cfg.pat

        def scale_chunk(eng, c, cl, Gs, Gt, wt):
            if eng == "a":
                for h in range(3):
                    sl = slice(129 * h, 129 * h + 129)
                    nc.scalar.activation(Gs[:, sl], Gt[:, cl, sl].opt(),
                                         Act.Copy,
                                         scale=wt[:, cl, h:h + 1].opt())
            else:
                dst = Gs[:, 0:387].rearrange("p (h c) -> p h c", h=3)
                src = Gt[:, cl, 0:387].rearrange("p (h c) -> p h c", h=3)
                wb = wt[:, cl, :].unsqueeze(-1).broadcast_to([128, 3, 129])
                if eng == "v":
                    nc.vector.tensor_tensor(dst, src, wb, Alu.mult)
                else:
                    nc.gpsimd.tensor_tensor(dst, src, wb, Alu.mult)

        # Dense-phase chunk runs per all-gather piece: a node chunk becomes
        # computable once every piece covering it has landed; chunks fully
        # inside one piece's region go to that piece, stragglers go last.
        assigned = np.full(NT, P - 1, np.int64)
        for p in range(P):
            for j in range(gpb):
                glo = j * SH + pieces[p][2]
                ghi = glo + pieces[p][3]
                lo = -(-glo // 128)
                hi = ghi // 128
                assigned[lo:hi] = p

        def runs_of(p):
            runs, s = [], None
            for t in range(NT):
                if assigned[t] == p and s is None:
                    s = t
                elif assigned[t] != p and s is not None:
                    runs.append((s, t))
                    s = None
            if s is not None:
                runs.append((s, NT))
            return runs

        piece_runs = [runs_of(p) for p in range(P)]

        def load_x(layer, xin, t, tb):
            """Fill xin[:, 0:tb, :] with x^T nodes [t*128, (t+tb)*128),
            reading xfull (layer 0) or the all-gather piece tensors."""
            a, bnd = t * 128, (t + tb) * 128
            flat = xin[:, 0:tb, :].rearrange("p c n -> p (c n)")
            if layer == 0:
                nc.sync.dma_start(flat, xfull.ap()[:, a:bnd])
                return
            for glo, ghi, tens, j in xpieces:
                lo, hi = max(a, glo), min(bnd, ghi)
                if lo < hi:
                    nc.sync.dma_start(flat[:, lo - a:hi - a],
                                      tens.ap()[j][:, lo - glo:hi - glo])

        def dense_gen(layer, runs, in_edge=False):
            """Generator emitting the fp8 z_aug table build for `runs` of
            node chunks; yields after each staged chunk. in_edge: emitted
            interleaved with the edge phase, where Pool is saturated with
            gather descriptor generation -- keep memsets off Pool there."""
            wa = waug1 if layer == 0 else waugn
            tab = zaug2[layer % 2]
            for r0_, r1_ in runs:
                t = r0_
                while t < r1_:
                    tb = min(TB, r1_ - t)
                    xin = xin_pool.tile([128, TB, 128], bf16, tag="xin")
                    load_x(layer, xin, t, tb)
                    s0 = 0
                    while s0 < tb:
                        sb = min(BST, tb - s0)
                        zt = zst_pool.tile([128, BST, ROW], f8, tag="zt")
                        ztb = zt.bitcast(bf16)
                        for s in range(sb):
                            psz = psz_pool.tile([128, 390], f32, tag="psz")
                            nc.tensor.matmul(psz[:], xin[:, s0 + s, :].opt(),
                                             wa[:], start=True, stop=True)
                            nc.gpsimd.memset(zt[:, s, 128:387:129], 1.0)
                            if cfg.stg_split and not in_edge:
                                # both engines stage each chunk (shorter psz
                                # hold): Act heads 0-1, DVE head 2 + el/er
                                nc.scalar.activation(
                                    zt[:, s, 0:258]
                                    .rearrange("p (g c) -> p g c", g=2)
                                    [:, :, 0:128],
                                    psz[:, 0:256]
                                    .rearrange("p (g c) -> p g c", g=2),
                                    Act.Copy)
                                nc.vector.tensor_copy(zt[:, s, 258:386],
                                                      psz[:, 256:384])
                                nc.vector.tensor_copy(ztb[:, s, 194:200],
                                                      psz[:, 384:390])
                                continue_yield = None
                            else:
                                zdst = (zt[:, s, 0:387]
                                        .rearrange("p (g c) -> p g c", g=3)
                                        [:, :, 0:128])
                                zsrc = (psz[:, 0:384]
                                        .rearrange("p (g c) -> p g c", g=3))
                                if (t + s0 + s) % 2:
                                    nc.scalar.activation(zdst, zsrc, Act.Copy)
                                    nc.vector.tensor_copy(ztb[:, s, 194:200],
                                                          psz[:, 384:390])
                                else:
                                    nc.vector.tensor_copy(zdst, zsrc)
                                    nc.scalar.activation(ztb[:, s, 194:200],
                                                         psz[:, 384:390],
                                                         Act.Copy)
                            yield
                        r0 = (t + s0) * 128
                        nc.sync.dma_start(
                            tab.ap()[r0:r0 + sb * 128, :]
                            .rearrange("(c p) z -> p c z", p=128),
                            zt[:, 0:sb, :])
                        s0 += sb
                    t += tb

        for _ in dense_gen(0, [(0, NT)]):
            pass

        # edge-phase-only constants load during the layer-0 dense phase so
        # they don't delay its first x loads on the SP queue
        nc.sync.dma_start(wl3[:], wl3_d.ap().rearrange("k p m -> p k m"))
        nc.sync.dma_start(blp1[:], blp1_d.ap())
        nc.sync.dma_start(iota[:], iota_d.ap())
        nc.sync.dma_start(ident[:], ident_d.ap())
        nc.sync.dma_start(dst3[:], dst3_d.ap())
        nc.sync.dma_start(zidx[:], zidx_d.ap())
        nc.sync.dma_start(eidx[:], eidx_d.ap())

        for layer in range(3):
            zaug = zaug2[layer % 2]
            # next layer's dense work, one generator per landed gather piece,
            # interleaved into this layer's edge phase
            if layer < 2:
                ogens = [[bcuts[p + 1] + 1,
                          dense_gen(layer + 1, piece_runs[p], in_edge=True)]
                         for p in range(P - 1)]
            else:
                ogens = []
            tail_done = [-1]

            # ---------------- edge phase ------------------------------------
            # Software pipeline: gathers issue GLA groups ahead and the
            # attention-weight chain (add/lrelu/exp) WLA groups ahead of the
            # group whose chunks are being consumed, so in-order engine
            # streams never sit at a semaphore for data that could have been
            # requested earlier.
            n_groups = -(-TOT // GC)
            GLA, WLA = cfg.GLA, cfg.WLA
            inflight = {}

            def issue_gathers(g):
                c0 = g * GC
                gsz = min(GC, TOT - c0)
                Gt = g_pool.tile([128, GC, ROW], f8, tag="G")
                R = r_pool.tile([128, GC, 256], f8, tag="R")
                nc.gpsimd.dma_gather(
                    Gt[:, 0:gsz, :], zaug.ap(),
                    zidx[:, 8 * c0: 8 * (c0 + gsz)],
                    num_idxs=gsz * 128, num_idxs_reg=gsz * 128,
                    elem_size=ROW, elem_step=ROW)
                nc.gpsimd.dma_gather(
                    R[:, 0:gsz, :], zaug.ap()[:, 256:512],
                    eidx[:, 8 * c0: 8 * (c0 + gsz)],
                    num_idxs=gsz * 128, num_idxs_reg=gsz * 128,
                    elem_size=256, elem_step=ROW)
                inflight[g] = [Gt, R, None]

            def issue_wt(g):
                c0 = g * GC
                gsz = min(GC, TOT - c0)
                Gt, R, _ = inflight[g]
                Gtb = Gt.bitcast(bf16)       # [128, GC, 256]
                Rb = R.bitcast(bf16)         # [128, GC, 128]
                wt = w_pool.tile([128, GC, 3], f32, tag="wt")
                nc.vector.tensor_tensor(
                    wt[:, 0:gsz, :], Gtb[:, 0:gsz, 194:197],
                    Rb[:, 0:gsz, 69:72], Alu.add)
                nc.vector.scalar_tensor_tensor(
                    wt[:, 0:gsz, :], wt[:, 0:gsz, :], cfg.neg_slope,
                    wt[:, 0:gsz, :], Alu.mult, Alu.max)
                nc.scalar.activation(wt[:, 0:gsz, :], wt[:, 0:gsz, :], Act.Exp)
                inflight[g][2] = wt

            for g in range(min(GLA, n_groups)):
                issue_gathers(g)
            for g in range(min(WLA, n_groups)):
                issue_wt(g)
            psb = None
            for gi in range(n_groups):
                if gi + GLA < n_groups:
                    issue_gathers(gi + GLA)
                if gi + WLA < n_groups:
                    issue_wt(gi + WLA)
                pulled = 0
                for og in ogens:
                    if og[1] is None or tail_done[0] < og[0]:
                        continue
                    while pulled < cfg.pull:
                        if next(og[1], "done") == "done":
                            og[1] = None
                            break
                        pulled += 1
                    if pulled >= cfg.pull:
                        break
                c0 = gi * GC
                gsz = min(GC, TOT - c0)
                Gt, R, wt = inflight.pop(gi)
                for cl in range(gsz):
                    c = c0 + cl
                    b = int(blk_of[c])
                    first = c == int(cum[b])
                    last = c == int(cum[b + 1]) - 1
                    if first:
                        psb = psb_pool.tile([128, 387], f32, tag="psb")
                    eng = cfg.scale_eng
                    if eng == "bal":
                        lp = (cfg.pat2 if layer == 2 and cfg.pat2 else _PAT)
                        eng = lp[c % len(lp)]
                    if eng == "o":
                        if cfg.lh_split:
                            lhs = []
                            for h in range(3):
                                lht = u_pool.tile([128, 128], bf16,
                                                  tag="u", name=f"lh{h}")
                                lhs.append(lht)
                        else:
                            lh3 = gs_pool.tile([128, 3, 128], bf16, tag="Gs")
                            lhs = [lh3[:, h, :].opt() for h in range(3)]
                        for h in range(3):
                            lh = lhs[h][:] if cfg.lh_split else lhs[h]
                            nc.vector.tensor_scalar(
                                lh, iota[:], dst3[:, c:c + 1],
                                wt[:, cl, h:h + 1].opt(),
                                Alu.is_equal, Alu.mult)
                        for h in range(3):
                            sl = slice(129 * h, 129 * h + 129)
                            lh = lhs[h][:] if cfg.lh_split else lhs[h]
                            nc.tensor.matmul(psb[:, sl], lh,
                                             Gt[:, cl, sl].opt(),
                                             start=(first and h == 0),
                                             stop=(last and h == 2))
                    else:
                        u = u_pool.tile([128, 128], bf16, tag="u")
                        if eng == "a":
                            nc.gpsimd.tensor_scalar(u[:], iota[:],
                                                    dst3[:, c:c + 1],
                                                    None, Alu.is_equal)
                        else:
                            nc.vector.tensor_scalar(u[:], iota[:],
                                                    dst3[:, c:c + 1],
                                                    None, Alu.is_equal)
                        Gs = gs_pool.tile([128, 390], bf16, tag="Gs")
                        scale_chunk(eng, c, cl, Gs, Gt, wt)
                        nc.tensor.matmul(psb[:, 0:387], u[:], Gs[:, 0:387],
                                         start=first, stop=last)
                    if last:
                        # ---------------- block tail ------------------------
                        r3 = s_pool.tile([128, 3], f32, tag="r3")
                        nc.vector.reciprocal(r3[:], psb[:, 128:387:129])
                        agg = a_pool.tile([128, 3, 128], bf16, tag="agg")
                        for h in range(3):
                            nc.scalar.activation(
                                agg[:, h, :].opt(),
                                psb[:, 129 * h:129 * h + 128],
                                Act.Copy, scale=r3[:, h:h + 1].opt())
                        aggT = at_pool.tile([128, 3, 128], bf16, tag="aggT")
                        for h in range(3):
                            pst = pst_pool.tile([128, 128], bf16, tag="pst")
                            nc.tensor.transpose(pst[:], agg[:, h, :].opt(),
                                                ident[:])
                            nc.vector.tensor_copy(aggT[:, h, :].opt(), pst[:])
                        bw = min(128, SH - b * 128)
                        psx = psx_pool.tile([128, 128], f32, tag="psx")
                        if layer < 2:
                            for k in range(3):
                                nc.tensor.matmul(psx[:], wl3[:, k, :].opt(),
                                                 aggT[:, k, :].opt(),
                                                 start=(k == 0), stop=(k == 2))
                            xsb = x_pool.tile([128, 128], bf16, tag="xsb")
                            nc.scalar.activation(xsb[:], psx[:], Act.Identity,
                                                 bias=blp1[:])
                            p = int(piece_of_block[b])
                            o = b * 128 - pieces[p][2]
                            nc.sync.dma_start(
                                xshp[p].ap()[:, o:o + bw], xsb[:, 0:bw])
                            if p < P - 1 and b == bcuts[p + 1] - 1:
                                do_gather(nc, p)
                            tail_done[0] = b
                        else:
                            for k in range(3):
                                nc.tensor.matmul(psx[:], aggT[:, k, :].opt(),
                                                 wl3[:, k, :].opt(),
                                                 start=(k == 0), stop=(k == 2))
                            x3 = x_pool.tile([128, 128], bf16, tag="xsb")
                            nc.vector.tensor_copy(x3[:], psx[:])
                            pw = pw_pool.tile([128, 128], bf16, tag="pw")
                            nc.sync.dma_start(pw[:], poolw_d.ap()[b])
                            nc.tensor.matmul(ps_pool_acc[:], pw[:], x3[:],
                                             start=(b == 0), stop=(b == NB - 1))

            for og in ogens:
                if og[1] is not None:
                    for _ in og[1]:
                        pass
            if layer < 2:
                do_gather(nc, P - 1)
                if layer == 0:
                    nc.sync.dma_start(blp1[:], blpn_d.ap())
                for _ in dense_gen(layer + 1, piece_runs[P - 1]):
                    pass

        po = x_pool.tile([128, 128], f32, tag="po")
        nc.vector.tensor_copy(po[:], ps_pool_acc[:])
        nc.sync.dma_start(pool_out.ap(), po[:])

        _rel_pst = (pst_pool,) if cfg.pst_bufs else ()
        for p in (pp_pool, pw_pool, x_pool, psx_pool, at_pool, *_rel_pst,
                  a_pool, s_pool, psb_pool, gs_pool, u_pool, w_pool,
                  r_pool, g_pool, zst_pool, psz_pool, xin_pool, cpool):
            p.release()

    nc.compile()
    return nc


# ---------------------------------------------------------------------------
# top-level kernel
# ---------------------------------------------------------------------------

def _prepare(cfg, inputs):
    """Returns (nc_b, in_maps, host_meta)."""
    import ml_dtypes
    bf = ml_dtypes.bfloat16
    npf = np.asarray
    per_core_edges = []
    nc_b = np.zeros(cfg.NB, np.int64)
    for br, (s, d) in enumerate((("srcA", "dstA"), ("srcB", "dstB"))):
        src = npf(inputs[s]).astype(np.int64)
        dst = npf(inputs[d]).astype(np.int64)
        for q in range(cfg.gpb):
            es, ed, cnt, lo = _prep_edges(cfg, src, dst, q)
            per_core_edges.append((es, ed, lo))
            nc_b = np.maximum(nc_b, -(-cnt // 128))
    in_maps = []
    host_meta = {}
    iota = np.tile(np.arange(128, dtype=bf), (128, 1))
    ident = np.eye(128, dtype=bf)
    for br in range(2):
        sfx = "AB"[br]
        W1 = npf(inputs["W1" + sfx]); al1 = npf(inputs["al1" + sfx])
        ar1 = npf(inputs["ar1" + sfx]); b1 = npf(inputs["b1" + sfx])
        Wn = npf(inputs["Wn" + sfx]); aln = npf(inputs["aln" + sfx])
        arn = npf(inputs["arn" + sfx]); bn = npf(inputs["bn" + sfx])
        Wl = npf(inputs["Wl" + sfx]); bl = npf(inputs["bl" + sfx])
        gid = npf(inputs["gid" + sfx]).astype(np.int64)
        feats = npf(inputs["feats" + sfx]).astype(np.float32)
        waug1, waugn, wl3, blp1, blpn = _prep_branch_weights(
            cfg, W1, al1, ar1, b1, Wn, aln, arn, bn, Wl, bl)
        xfull = np.zeros((128, cfg.NTP), bf)
        xfull[:, :cfg.N] = feats.T.astype(bf)
        host_meta[sfx] = dict(blpn=blpn, gid=gid)
        for q in range(cfg.gpb):
            es, ed, lo = per_core_edges[br * cfg.gpb + q]
            zidx, eidx, dst3 = _pack_core(cfg, es, ed, lo, nc_b)
            poolw = np.zeros((cfg.NB, 128, 128), bf)
            for b in range(cfg.NB):
                for i in range(min(128, cfg.SH - b * 128)):
                    n = lo + b * 128 + i
                    if n < cfg.N:
                        poolw[b, i, gid[n]] = 1.0
            in_maps.append({
                "xfull": xfull,
                "waug1": waug1.astype(bf), "waugn": waugn.astype(bf),
                "wl3": wl3.astype(bf), "blp1": blp1.reshape(128, 1),
                "blpn": blpn.reshape(128, 1),
                "iota": iota, "ident": ident,
                "dst3": dst3, "zidx": zidx, "eidx": eidx, "poolw": poolw,
            })
    return nc_b, in_maps, host_meta


def _finalize(cfg, inputs, host_meta, pool_outs):
    """pool_outs: list of 8 [128,128] arrays -> full output [G,1] float64."""
    out = {}
    for br in range(2):
        sfx = "AB"[br]
        total = np.zeros((128, 128), np.float64)
        for q in range(cfg.gpb):
            total += pool_outs[br * cfg.gpb + q].astype(np.float64)
        gid = host_meta[sfx]["gid"]
        cnt = np.bincount(gid, minlength=128).astype(np.float64)
        total += cnt[:, None] * host_meta[sfx]["blpn"].astype(np.float64)[None, :]
        out[sfx] = (total / np.maximum(cnt[:, None], 1.0))[:cfg.G]
    cat = np.concatenate([out["A"], out["B"]], axis=1)
    Wo = np.asarray(inputs["Wo"]).astype(np.float64)
    bo = np.asarray(inputs["bo"]).astype(np.float64)
    return (cat @ Wo + bo).astype(np.float64)


_CACHE = {}


def kernel(**inputs):
    cfg = Cfg(N=inputs["featsA"].shape[0], G=128)
    nc_b, in_maps, host_meta = _prepare(cfg, inputs)
    key = ("prog", tuple(nc_b.tolist()))
    if key not in _CACHE:
        _CACHE[key] = build_program(cfg, nc_b)
    nc = _CACHE[key]
    from concourse.bass_utils import run_bass_kernel_spmd
    res = run_bass_kernel_spmd(nc, in_maps, list(range(cfg.n_cores)))
    pool_outs = [r["pool_out"] for r in res.results]
    return _finalize(cfg, inputs, host_meta, pool_outs)


# revision 87
# speedup vs baseline: 1.0070x; 1.0025x over previous
"""Trainium2 Bass kernel for nn_DoubleNet (two GATNet branches + avg-pool + linear).

Strategy (8 NeuronCores):
  - Cores 0-3 run branch A, cores 4-7 run branch B (same SPMD program,
    different input data per core). Within a branch, dst nodes are sharded
    contiguously across the 4 cores.
  - Per GAT layer:
      dense phase: stream x^T (bf16) from the all-gather piece tensors,
        compute z_aug = x @ [W|W@al|W@ar] on the PE (bf16 weights), and stage
        each 128-node chunk into a 512-byte table row: [z0|1|z1|1|z2|1] as
        fp8 in bytes 0:387, el/er as bf16 in bytes 388:400. Two DRAM tables
        ping-pong across layers so the next layer's dense phase can overlap
        the current edge phase.
      edge phase: edges are pre-sorted by dst (host side) and processed in
        chunks of 128, gathered 8 chunks per dma_gather call (the 1024-entry
        SWDGE ring bounds the call size): the full 512B row of each src
        (fp8 z + bf16 el), and the 256B second half-row of each dst (er).
        Gathers issue GLA groups ahead and the attention-weight chain
        (add + leaky_relu on DVE, exp on Act) WLA groups ahead of
        consumption. Per chunk, one of three engine-balanced aggregation
        styles ("o": 3 fused scaled one-hots on DVE + 3 PE matmuls against
        the raw fp8 rows; "v": unscaled one-hot + one broadcast
        tensor_tensor scale on DVE + 1 matmul; "a": one-hot on Pool + 3
        scaled copies on Act + 1 matmul) accumulates messages and softmax
        denominators (the interleaved ones columns) into a per-dst-block
        PSUM tile.
      block tail: denominator reciprocal (DVE), normalize to bf16 (Act),
        PE-transpose, evacuate (DVE), x_next^T = Wl^T @ agg^T (PE), bias
        (Act). Per-core x^T shards all-gather in three pieces; each piece
        triggers the next layer's dense work for the node chunks it covers,
        interleaved into the remaining edge-phase emission.
  - Final layer pools via a host-precomputed gid one-hot matmul; host sums the
    per-core partial pools and applies the output linear (float64).
"""

import sys

sys.path.insert(0, "/opt/trn_rl_repo")

import numpy as np


# ---------------------------------------------------------------------------
# configuration
# ---------------------------------------------------------------------------

class Cfg:
    def __init__(self, N=20000, G=128, H=3, EMB=128, F=128, n_cores=8,
                 neg_slope=0.2, GC=8, BST=4, TB=8, scale_eng="bal"):
        assert F == 128 and EMB == 128 and H == 3
        self.N, self.G, self.H, self.EMB, self.F = N, G, H, EMB, F
        self.n_cores = n_cores
        self.gpb = n_cores // 2            # cores per branch
        assert N % self.gpb == 0
        self.SH = N // self.gpb            # dst nodes per core
        self.NB = -(-self.SH // 128)       # dst blocks per core
        self.NT = -(-N // 128)             # node chunks for dense phase
        self.NTP = self.NT * 128           # padded node count
        self.neg_slope = neg_slope
        self.GC = GC                       # chunks per z-gather call
        self.BST = BST                     # chunks staged per table write
        self.TB = TB                       # chunks per dense x load
        self.scale_eng = scale_eng         # engines for per-head w-scaling
        self.ROW = 512                     # table row bytes (fp8 elements)
        self.dma_scratch = 16384           # SWDGE ring carveout bytes/partition
        self.psz_bufs = 3
        self.psb_bufs = 2
        self.pst_bufs = 1
        self.psx_bufs = 1
        self.GLA = 4                       # gather lookahead (groups)
        self.WLA = 2                       # attention-weight chain lookahead
        self.pat = "ooavooaoooavooaoo"
        self.pat2 = None                   # layer-2 override (no overlap work)
        self.pull = 5                      # dense chunks interleaved per group
        self.cuts = (22, 31)               # all-gather piece block boundaries
        self.zst_bufs = 12
        self.xin_bufs = 4
        self.g_bufs = 9
        self.gs_bufs = 18
        self.stg_split = False
        self.lh_split = False
        self.u_bufs = 24


# ---------------------------------------------------------------------------
# host-side data prep
# ---------------------------------------------------------------------------

def _prep_edges(cfg, src, dst, q):
    """Edges of one core (dst in its shard), dst-sorted, fake rows added."""
    lo = q * cfg.SH
    sel = (dst >= lo) & (dst < lo + cfg.SH)
    es = src[sel].astype(np.int64)
    ed = (dst[sel].astype(np.int64) - lo)
    nfake = cfg.NB * 128 - cfg.SH
    if nfake:
        es = np.concatenate([es, np.zeros(nfake, np.int64)])
        ed = np.concatenate([ed, np.arange(cfg.SH, cfg.NB * 128, dtype=np.int64)])
    order = np.argsort(ed, kind="stable")
    es, ed = es[order], ed[order]
    cnt = np.bincount(ed // 128, minlength=cfg.NB)
    return es, ed, cnt, lo


def _pack_core(cfg, es, ed, lo, nc_b):
    """Build flat (block, chunk, slot) arrays padded to nc_b chunks/block."""
    TOT = int(nc_b.sum())
    zsrc = np.zeros(TOT * 128, np.int16)
    edst = np.zeros(TOT * 128, np.int16)
    dst3 = np.full(TOT * 128, -1.0, np.float32)
    epos = np.searchsorted(ed, np.arange(0, cfg.NB * 128 + 1, 128))
    cum = np.concatenate([[0], np.cumsum(nc_b)]).astype(int)
    for b in range(cfg.NB):
        s, e = epos[b], epos[b + 1]
        n = e - s
        o = int(cum[b]) * 128
        zsrc[o:o + n] = es[s:e]
        # fake rows (local id >= SH) must not use an out-of-range er index
        ei = ed[s:e] + lo
        ei[ed[s:e] >= cfg.SH] = 0
        edst[o:o + n] = ei
        dst3[o:o + n] = (ed[s:e] - b * 128).astype(np.float32)
    # index tiles: flat i -> (partition i%16, col i//16), replicated to 128 rows
    def wrap(a):
        return np.tile(a.reshape(-1, 16).T, (8, 1)).copy()
    # dst3 partition-major: [128, TOT]
    d3 = dst3.reshape(TOT, 128).T.copy()
    return wrap(zsrc), wrap(edst), d3


def _prep_branch_weights(cfg, W1, al1, ar1, b1, Wn, aln, arn, bn, Wl, bl):
    H, EMB = cfg.H, cfg.EMB

    def waug(W, al, ar):
        K = W.shape[0]
        out = np.zeros((K, 390), np.float32)
        out[:, :384] = W
        for h in range(H):
            out[:, 384 + h] = W[:, h * EMB:(h + 1) * EMB] @ al[h]
            out[:, 387 + h] = W[:, h * EMB:(h + 1) * EMB] @ ar[h]
        return out

    wl3 = Wl.reshape(3, 128, EMB).astype(np.float32)
    blp1 = (b1 @ Wl + bl).astype(np.float32)
    blpn = (bn @ Wl + bl).astype(np.float32)
    return waug(W1, al1, ar1), waug(Wn, aln, arn), wl3, blp1, blpn


# ---------------------------------------------------------------------------
# device program
# ---------------------------------------------------------------------------

def build_program(cfg, nc_b, timing_mode=False):
    import concourse.bass as bass
    import concourse.mybir as mybir
    import concourse.tile as tile
    from concourse import bacc

    dt = mybir.dt
    f32 = dt.float32
    bf16 = dt.bfloat16
    f8 = dt.float8e4
    Alu = mybir.AluOpType
    Act = mybir.ActivationFunctionType

    NB, NT, SH, GC, BST, TB = cfg.NB, cfg.NT, cfg.SH, cfg.GC, cfg.BST, cfg.TB
    ROW = cfg.ROW
    TOT = int(nc_b.sum())
    cum = np.concatenate([[0], np.cumsum(nc_b)]).astype(int)
    # block index of each chunk
    blk_of = np.zeros(TOT, np.int64)
    for b in range(NB):
        blk_of[cum[b]:cum[b + 1]] = b
    gpb = cfg.gpb
    groups = [list(range(gpb)), list(range(gpb, 2 * gpb))]

    nc = bacc.Bacc("TRN2", target_bir_lowering=False, debug=False,
                   num_devices=cfg.n_cores,
                   dynamic_dma_scratch_size=cfg.dma_scratch)

    # inputs -----------------------------------------------------------------
    xfull = nc.dram_tensor("xfull", [128, cfg.NTP], bf16, kind="ExternalInput")
    waug1_d = nc.dram_tensor("waug1", [128, 390], bf16, kind="ExternalInput")
    waugn_d = nc.dram_tensor("waugn", [128, 390], bf16, kind="ExternalInput")
    wl3_d = nc.dram_tensor("wl3", [3, 128, 128], bf16, kind="ExternalInput")
    blp1_d = nc.dram_tensor("blp1", [128, 1], f32, kind="ExternalInput")
    blpn_d = nc.dram_tensor("blpn", [128, 1], f32, kind="ExternalInput")
    iota_d = nc.dram_tensor("iota", [128, 128], bf16, kind="ExternalInput")
    ident_d = nc.dram_tensor("ident", [128, 128], bf16, kind="ExternalInput")
    dst3_d = nc.dram_tensor("dst3", [128, TOT], f32, kind="ExternalInput")
    zidx_d = nc.dram_tensor("zidx", [128, TOT * 8], dt.int16, kind="ExternalInput")
    eidx_d = nc.dram_tensor("eidx", [128, TOT * 8], dt.int16, kind="ExternalInput")
    poolw_d = nc.dram_tensor("poolw", [NB, 128, 128], bf16, kind="ExternalInput")
    pool_out = nc.dram_tensor("pool_out", [128, 128], f32, kind="ExternalOutput")

    # internal DRAM ----------------------------------------------------------
    # Two z-tables ping-pong so layer L+1's dense phase (for the SH1 node
    # regions that all-gather mid-edge-phase) can overlap layer L's edge
    # phase without clobbering rows its gathers still read.
    zaug2 = [nc.dram_tensor("zaugA", [cfg.NTP, ROW], f8),
             nc.dram_tensor("zaugB", [cfg.NTP, ROW], f8)]
    # The per-core x^T shard all-gathers in P pieces so the next layer's
    # dense phase can start on each piece as soon as it lands.
    bcuts = [0, *cfg.cuts, NB]
    P = len(bcuts) - 1
    pieces = []                            # (b0, b1, col0, ncols)
    for p in range(P):
        b0, b1 = bcuts[p], bcuts[p + 1]
        col0 = b0 * 128
        ncols = min(b1 * 128, SH) - col0
        pieces.append((b0, b1, col0, ncols))
    xshp = [nc.dram_tensor(f"xsh{p}", [128, pieces[p][3]], bf16)
            for p in range(P)]
    xgathp = [nc.dram_tensor(f"xgath{p}", [gpb, 128, pieces[p][3]], bf16)
              for p in range(P)]

    def do_gather(nc, p):
        xs, xg = xshp[p], xgathp[p]
        if timing_mode:
            for j in range(gpb):
                nc.sync.dma_start(xg.ap()[j], xs.ap())
        else:
            nc.gpsimd.collective_compute(
                "AllGather", mybir.AluOpType.bypass, replica_groups=groups,
                ins=[xs.ap()], outs=[xg.ap()])

    # node-interval -> all-gather piece map (for dense-phase x loads)
    xpieces = []
    for j in range(gpb):
        for p in range(P):
            glo = j * SH + pieces[p][2]
            xpieces.append((glo, glo + pieces[p][3], xgathp[p], j))

    piece_of_block = np.zeros(NB, np.int64)
    for p in range(P):
        piece_of_block[bcuts[p]:bcuts[p + 1]] = p

    with tile.TileContext(nc) as tc:
        cpool = tc.alloc_tile_pool(name="const", bufs=1)
        waug1 = cpool.tile([128, 390], bf16, tag="waug1")
        waugn = cpool.tile([128, 390], bf16, tag="waugn")
        wl3 = cpool.tile([128, 3, 128], bf16, tag="wl3")
        blp1 = cpool.tile([128, 1], f32, tag="blp1")
        iota = cpool.tile([128, 128], bf16, tag="iota")
        ident = cpool.tile([128, 128], bf16, tag="ident")
        dst3 = cpool.tile([128, TOT], f32, tag="dst3")
        zidx = cpool.tile([128, TOT * 8], dt.int16, tag="zidx")
        eidx = cpool.tile([128, TOT * 8], dt.int16, tag="eidx")

        nc.sync.dma_start(waug1[:], waug1_d.ap())
        nc.sync.dma_start(waugn[:], waugn_d.ap())

        xin_pool = tc.alloc_tile_pool(name="xin", bufs=cfg.xin_bufs)
        psz_pool = tc.alloc_tile_pool(name="psz", bufs=cfg.psz_bufs, space="PSUM")
        zst_pool = tc.alloc_tile_pool(name="zst", bufs=cfg.zst_bufs)
        g_pool = tc.alloc_tile_pool(name="g", bufs=cfg.g_bufs)
        r_pool = tc.alloc_tile_pool(name="r", bufs=cfg.g_bufs)
        w_pool = tc.alloc_tile_pool(name="w", bufs=6)
        u_pool = tc.alloc_tile_pool(name="u", bufs=cfg.u_bufs)
        gs_pool = tc.alloc_tile_pool(name="gs", bufs=cfg.gs_bufs)
        psb_pool = tc.alloc_tile_pool(name="psb", bufs=cfg.psb_bufs, space="PSUM")
        s_pool = tc.alloc_tile_pool(name="s", bufs=2)
        a_pool = tc.alloc_tile_pool(name="a", bufs=2)
        if cfg.pst_bufs:
            pst_pool = tc.alloc_tile_pool(name="pst", bufs=cfg.pst_bufs,
                                          space="PSUM")
        at_pool = tc.alloc_tile_pool(name="at", bufs=2)
        psx_pool = tc.alloc_tile_pool(name="psx", bufs=cfg.psx_bufs,
                                      space="PSUM")
        if not cfg.pst_bufs:
            pst_pool = psx_pool        # transposes share the psx banks
        x_pool = tc.alloc_tile_pool(name="x", bufs=2)
        pw_pool = tc.alloc_tile_pool(name="pw", bufs=2)
        pp_pool = tc.alloc_tile_pool(name="pp", bufs=1, space="PSUM")

        ps_pool_acc = pp_pool.tile([128, 128], f32, tag="poolacc")

        # Per-chunk aggregation styles, rotated to balance engines:
        #   "o": 3 fused scaled-one-hots on DVE (is_equal*w) + 3 PE matmuls
        #        against the raw fp8 rows -- cheap on DVE, heavy on PE SEQ.
        #   "v": unscaled one-hot + ONE fused broadcast tensor_tensor scale
        #        on DVE + 1 PE matmul.
        #   "a": unscaled one-hot (Pool) + 3 scaled copies on Act + 1 matmul.
        # Pool otherwise only generates gather descriptors (its in-order
        # sequencer must not block behind data-dependent work).
        _PAT = cfg.pat

        def scale_chunk(eng, c, cl, Gs, Gt, wt):
            if eng == "a":
                for h in range(3):
                    sl = slice(129 * h, 129 * h + 129)
                    nc.scalar.activation(Gs[:, sl], Gt[:, cl, sl].opt(),
                                         Act.Copy,
                                         scale=wt[:, cl, h:h + 1].opt())
            else:
                dst = Gs[:, 0:387].rearrange("p (h c) -> p h c", h=3)
                src = Gt[:, cl, 0:387].rearrange("p (h c) -> p h c", h=3)
                wb = wt[:, cl, :].unsqueeze(-1).broadcast_to([128, 3, 129])
                if eng == "v":
                    nc.vector.tensor_tensor(dst, src, wb, Alu.mult)
                else:
                    nc.gpsimd.tensor_tensor(dst, src, wb, Alu.mult)

        # Dense-phase chunk runs per all-gather piece: a node chunk becomes
        # computable once every piece covering it has landed; chunks fully
        # inside one piece's region go to that piece, stragglers go last.
        assigned = np.full(NT, P - 1, np.int64)
        for p in range(P):
            for j in range(gpb):
                glo = j * SH + pieces[p][2]
                ghi = glo + pieces[p][3]
                lo = -(-glo // 128)
                hi = ghi // 128
                assigned[lo:hi] = p

        def runs_of(p):
            runs, s = [], None
            for t in range(NT):
                if assigned[t] == p and s is None:
                    s = t
                elif assigned[t] != p and s is not None:
                    runs.append((s, t))
                    s = None
            if s is not None:
                runs.append((s, NT))
            return runs

        piece_runs = [runs_of(p) for p in range(P)]

        def load_x(layer, xin, t, tb):
            """Fill xin[:, 0:tb, :] with x^T nodes [t*128, (t+tb)*128),
            reading xfull (layer 0) or the all-gather piece tensors."""
            a, bnd = t * 128, (t + tb) * 128
            flat = xin[:, 0:tb, :].rearrange("p c n -> p (c n)")
            if layer == 0:
                nc.sync.dma_start(flat, xfull.ap()[:, a:bnd])
                return
            for glo, ghi, tens, j in xpieces:
                lo, hi = max(a, glo), min(bnd, ghi)
                if lo < hi:
                    nc.sync.dma_start(flat[:, lo - a:hi - a],
                                      tens.ap()[j][:, lo - glo:hi - glo])

        def dense_gen(layer, runs, in_edge=False):
            """Generator emitting the fp8 z_aug table build for `runs` of
            node chunks; yields after each staged chunk. in_edge: emitted
            interleaved with the edge phase, where Pool is saturated with
            gather descriptor generation -- keep memsets off Pool there."""
            wa = waug1 if layer == 0 else waugn
            tab = zaug2[layer % 2]
            for r0_, r1_ in runs:
                t = r0_
                while t < r1_:
                    tb = min(TB, r1_ - t)
                    xin = xin_pool.tile([128, TB, 128], bf16, tag="xin")
                    load_x(layer, xin, t, tb)
                    s0 = 0
                    while s0 < tb:
                        sb = min(BST, tb - s0)
                        zt = zst_pool.tile([128, BST, ROW], f8, tag="zt")
                        ztb = zt.bitcast(bf16)
                        for s in range(sb):
                            psz = psz_pool.tile([128, 390], f32, tag="psz")
                            nc.tensor.matmul(psz[:], xin[:, s0 + s, :].opt(),
                                             wa[:], start=True, stop=True)
                            nc.gpsimd.memset(zt[:, s, 128:387:129], 1.0)
                            if cfg.stg_split and not in_edge:
                                # both engines stage each chunk (shorter psz
                                # hold): Act heads 0-1, DVE head 2 + el/er
                                nc.scalar.activation(
                                    zt[:, s, 0:258]
                                    .rearrange("p (g c) -> p g c", g=2)
                                    [:, :, 0:128],
                                    psz[:, 0:256]
                                    .rearrange("p (g c) -> p g c", g=2),
                                    Act.Copy)
                                nc.vector.tensor_copy(zt[:, s, 258:386],
                                                      psz[:, 256:384])
                                nc.vector.tensor_copy(ztb[:, s, 194:200],
                                                      psz[:, 384:390])
                                continue_yield = None
                            else:
                                zdst = (zt[:, s, 0:387]
                                        .rearrange("p (g c) -> p g c", g=3)
                                        [:, :, 0:128])
                                zsrc = (psz[:, 0:384]
                                        .rearrange("p (g c) -> p g c", g=3))
                                if (t + s0 + s) % 2:
                                    nc.scalar.activation(zdst, zsrc, Act.Copy)
                                    nc.vector.tensor_copy(ztb[:, s, 194:200],
                                                          psz[:, 384:390])
                                else:
                                    nc.vector.tensor_copy(zdst, zsrc)
                                    nc.scalar.activation(ztb[:, s, 194:200],
                                                         psz[:, 384:390],
                                                         Act.Copy)
                            yield
                        r0 = (t + s0) * 128
                        nc.sync.dma_start(
                            tab.ap()[r0:r0 + sb * 128, :]
                            .rearrange("(c p) z -> p c z", p=128),
                            zt[:, 0:sb, :])
                        s0 += sb
                    t += tb

        for _ in dense_gen(0, [(0, NT)]):
            pass

        # edge-phase-only constants load during the layer-0 dense phase so
        # they don't delay its first x loads on the SP queue
        nc.sync.dma_start(wl3[:], wl3_d.ap().rearrange("k p m -> p k m"))
        nc.sync.dma_start(blp1[:], blp1_d.ap())
        nc.sync.dma_start(iota[:], iota_d.ap())
        nc.sync.dma_start(ident[:], ident_d.ap())
        nc.sync.dma_start(dst3[:], dst3_d.ap())
        nc.sync.dma_start(zidx[:], zidx_d.ap())
        nc.sync.dma_start(eidx[:], eidx_d.ap())

        for layer in range(3):
            zaug = zaug2[layer % 2]
            # next layer's dense work, one generator per landed gather piece,
            # interleaved into this layer's edge phase
            if layer < 2:
                ogens = [[bcuts[p + 1] + 1,
                          dense_gen(layer + 1, piece_runs[p], in_edge=True)]
                         for p in range(P - 1)]
            else:
                ogens = []
            tail_done = [-1]

            # ---------------- edge phase ------------------------------------
            # Software pipeline: gathers issue GLA groups ahead and the
            # attention-weight chain (add/lrelu/exp) WLA groups ahead of the
            # group whose chunks are being consumed, so in-order engine
            # streams never sit at a semaphore for data that could have been
            # requested earlier.
            n_groups = -(-TOT // GC)
            GLA, WLA = cfg.GLA, cfg.WLA
            inflight = {}

            def issue_gathers(g):
                c0 = g * GC
                gsz = min(GC, TOT - c0)
                Gt = g_pool.tile([128, GC, ROW], f8, tag="G")
                R = r_pool.tile([128, GC, 256], f8, tag="R")
                nc.gpsimd.dma_gather(
                    Gt[:, 0:gsz, :], zaug.ap(),
                    zidx[:, 8 * c0: 8 * (c0 + gsz)],
                    num_idxs=gsz * 128, num_idxs_reg=gsz * 128,
                    elem_size=ROW, elem_step=ROW)
                nc.gpsimd.dma_gather(
                    R[:, 0:gsz, :], zaug.ap()[:, 256:512],
                    eidx[:, 8 * c0: 8 * (c0 + gsz)],
                    num_idxs=gsz * 128, num_idxs_reg=gsz * 128,
                    elem_size=256, elem_step=ROW)
                inflight[g] = [Gt, R, None]

            def issue_wt(g):
                c0 = g * GC
                gsz = min(GC, TOT - c0)
                Gt, R, _ = inflight[g]
                Gtb = Gt.bitcast(bf16)       # [128, GC, 256]
                Rb = R.bitcast(bf16)         # [128, GC, 128]
                wt = w_pool.tile([128, GC, 3], f32, tag="wt")
                nc.vector.tensor_tensor(
                    wt[:, 0:gsz, :], Gtb[:, 0:gsz, 194:197],
                    Rb[:, 0:gsz, 69:72], Alu.add)
                nc.vector.scalar_tensor_tensor(
                    wt[:, 0:gsz, :], wt[:, 0:gsz, :], cfg.neg_slope,
                    wt[:, 0:gsz, :], Alu.mult, Alu.max)
                nc.scalar.activation(wt[:, 0:gsz, :], wt[:, 0:gsz, :], Act.Exp)
                inflight[g][2] = wt

            for g in range(min(GLA, n_groups)):
                issue_gathers(g)
            for g in range(min(WLA, n_groups)):
                issue_wt(g)
            psb = None
            for gi in range(n_groups):
                if gi + GLA < n_groups:
                    issue_gathers(gi + GLA)
                if gi + WLA < n_groups:
                    issue_wt(gi + WLA)
                pulled = 0
                for og in ogens:
                    if og[1] is None or tail_done[0] < og[0]:
                        continue
                    while pulled < cfg.pull:
                        if next(og[1], "done") == "done":
                            og[1] = None
                            break
                        pulled += 1
                    if pulled >= cfg.pull:
                        break
                c0 = gi * GC
                gsz = min(GC, TOT - c0)
                Gt, R, wt = inflight.pop(gi)
                for cl in range(gsz):
                    c = c0 + cl
                    b = int(blk_of[c])
                    first = c == int(cum[b])
                    last = c == int(cum[b + 1]) - 1
                    if first:
                        psb = psb_pool.tile([128, 387], f32, tag="psb")
                    eng = cfg.scale_eng
                    if eng == "bal":
                        lp = (cfg.pat2 if layer == 2 and cfg.pat2 else _PAT)
                        eng = lp[c % len(lp)]
                    if eng == "o":
                        if cfg.lh_split:
                            lhs = []
                            for h in range(3):
                                lht = u_pool.tile([128, 128], bf16,
                                                  tag="u", name=f"lh{h}")
                                lhs.append(lht)
                        else:
                            lh3 = gs_pool.tile([128, 3, 128], bf16, tag="Gs")
                            lhs = [lh3[:, h, :].opt() for h in range(3)]
                        for h in range(3):
                            lh = lhs[h][:] if cfg.lh_split else lhs[h]
                            nc.vector.tensor_scalar(
                                lh, iota[:], dst3[:, c:c + 1],
                                wt[:, cl, h:h + 1].opt(),
                                Alu.is_equal, Alu.mult)
                        for h in range(3):
                            sl = slice(129 * h, 129 * h + 129)
                            lh = lhs[h][:] if cfg.lh_split else lhs[h]
                            nc.tensor.matmul(psb[:, sl], lh,
                                             Gt[:, cl, sl].opt(),
                                             start=(first and h == 0),
                                             stop=(last and h == 2))
                    else:
                        u = u_pool.tile([128, 128], bf16, tag="u")
                        if eng == "a":
                            nc.gpsimd.tensor_scalar(u[:], iota[:],
                                                    dst3[:, c:c + 1],
                                                    None, Alu.is_equal)
                        else:
                            nc.vector.tensor_scalar(u[:], iota[:],
                                                    dst3[:, c:c + 1],
                                                    None, Alu.is_equal)
                        Gs = gs_pool.tile([128, 390], bf16, tag="Gs")
                        scale_chunk(eng, c, cl, Gs, Gt, wt)
                        nc.tensor.matmul(psb[:, 0:387], u[:], Gs[:, 0:387],
                                         start=first, stop=last)
                    if last:
                        # ---------------- block tail ------------------------
                        r3 = s_pool.tile([128, 3], f32, tag="r3")
                        nc.vector.reciprocal(r3[:], psb[:, 128:387:129])
                        agg = a_pool.tile([128, 3, 128], bf16, tag="agg")
                        for h in range(3):
                            nc.scalar.activation(
                                agg[:, h, :].opt(),
                                psb[:, 129 * h:129 * h + 128],
                                Act.Copy, scale=r3[:, h:h + 1].opt())
                        aggT = at_pool.tile([128, 3, 128], bf16, tag="aggT")
                        for h in range(3):
                            pst = pst_pool.tile([128, 128], bf16, tag="pst")
                            nc.tensor.transpose(pst[:], agg[:, h, :].opt(),
                                                ident[:])
                            nc.vector.tensor_copy(aggT[:, h, :].opt(), pst[:])
                        bw = min(128, SH - b * 128)
                        psx = psx_pool.tile([128, 128], f32, tag="psx")
                        if layer < 2:
                            for k in range(3):
                                nc.tensor.matmul(psx[:], wl3[:, k, :].opt(),
                                                 aggT[:, k, :].opt(),
                                                 start=(k == 0), stop=(k == 2))
                            xsb = x_pool.tile([128, 128], bf16, tag="xsb")
                            nc.scalar.activation(xsb[:], psx[:], Act.Identity,
                                                 bias=blp1[:])
                            p = int(piece_of_block[b])
                            o = b * 128 - pieces[p][2]
                            nc.sync.dma_start(
                                xshp[p].ap()[:, o:o + bw], xsb[:, 0:bw])
                            if p < P - 1 and b == bcuts[p + 1] - 1:
                                do_gather(nc, p)
                            tail_done[0] = b
                        else:
                            for k in range(3):
                                nc.tensor.matmul(psx[:], aggT[:, k, :].opt(),
                                                 wl3[:, k, :].opt(),
                                                 start=(k == 0), stop=(k == 2))
                            x3 = x_pool.tile([128, 128], bf16, tag="xsb")
                            nc.vector.tensor_copy(x3[:], psx[:])
                            pw = pw_pool.tile([128, 128], bf16, tag="pw")
                            nc.sync.dma_start(pw[:], poolw_d.ap()[b])
                            nc.tensor.matmul(ps_pool_acc[:], pw[:], x3[:],
                                             start=(b == 0), stop=(b == NB - 1))

            for og in ogens:
                if og[1] is not None:
                    for _ in og[1]:
                        pass
            if layer < 2:
                do_gather(nc, P - 1)
                if layer == 0:
                    nc.sync.dma_start(blp1[:], blpn_d.ap())
                for _ in dense_gen(layer + 1, piece_runs[P - 1]):
                    pass

        po = x_pool.tile([128, 128], f32, tag="po")
        nc.vector.tensor_copy(po[:], ps_pool_acc[:])
        nc.sync.dma_start(pool_out.ap(), po[:])

        _rel_pst = (pst_pool,) if cfg.pst_bufs else ()
        for p in (pp_pool, pw_pool, x_pool, psx_pool, at_pool, *_rel_pst,
                  a_pool, s_pool, psb_pool, gs_pool, u_pool, w_pool,
                  r_pool, g_pool, zst_pool, psz_pool, xin_pool, cpool):
            p.release()

    nc.compile()
    return nc


# ---------------------------------------------------------------------------
# top-level kernel
# ---------------------------------------------------------------------------

def _prepare(cfg, inputs):
    """Returns (nc_b, in_maps, host_meta)."""
    import ml_dtypes
    bf = ml_dtypes.bfloat16
    npf = np.asarray
    per_core_edges = []
    nc_b = np.zeros(cfg.NB, np.int64)
    for br, (s, d) in enumerate((("srcA", "dstA"), ("srcB", "dstB"))):
        src = npf(inputs[s]).astype(np.int64)
        dst = npf(inputs[d]).astype(np.int64)
        for q in range(cfg.gpb):
            es, ed, cnt, lo = _prep_edges(cfg, src, dst, q)
            per_core_edges.append((es, ed, lo))
            nc_b = np.maximum(nc_b, -(-cnt // 128))
    in_maps = []
    host_meta = {}
    iota = np.tile(np.arange(128, dtype=bf), (128, 1))
    ident = np.eye(128, dtype=bf)
    for br in range(2):
        sfx = "AB"[br]
        W1 = npf(inputs["W1" + sfx]); al1 = npf(inputs["al1" + sfx])
        ar1 = npf(inputs["ar1" + sfx]); b1 = npf(inputs["b1" + sfx])
        Wn = npf(inputs["Wn" + sfx]); aln = npf(inputs["aln" + sfx])
        arn = npf(inputs["arn" + sfx]); bn = npf(inputs["bn" + sfx])
        Wl = npf(inputs["Wl" + sfx]); bl = npf(inputs["bl" + sfx])
        gid = npf(inputs["gid" + sfx]).astype(np.int64)
        feats = npf(inputs["feats" + sfx]).astype(np.float32)
        waug1, waugn, wl3, blp1, blpn = _prep_branch_weights(
            cfg, W1, al1, ar1, b1, Wn, aln, arn, bn, Wl, bl)
        xfull = np.zeros((128, cfg.NTP), bf)
        xfull[:, :cfg.N] = feats.T.astype(bf)
        host_meta[sfx] = dict(blpn=blpn, gid=gid)
        for q in range(cfg.gpb):
            es, ed, lo = per_core_edges[br * cfg.gpb + q]
            zidx, eidx, dst3 = _pack_core(cfg, es, ed, lo, nc_b)
            poolw = np.zeros((cfg.NB, 128, 128), bf)
            for b in range(cfg.NB):
                for i in range(min(128, cfg.SH - b * 128)):
                    n = lo + b * 128 + i
                    if n < cfg.N:
                        poolw[b, i, gid[n]] = 1.0
            in_maps.append({
                "xfull": xfull,
                "waug1": waug1.astype(bf), "waugn": waugn.astype(bf),
                "wl3": wl3.astype(bf), "blp1": blp1.reshape(128, 1),
                "blpn": blpn.reshape(128, 1),
                "iota": iota, "ident": ident,
                "dst3": dst3, "zidx": zidx, "eidx": eidx, "poolw": poolw,
            })
    return nc_b, in_maps, host_meta


def _finalize(cfg, inputs, host_meta, pool_outs):
    """pool_outs: list of 8 [128,128] arrays -> full output [G,1] float64."""
    out = {}
    for br in range(2):
        sfx = "AB"[br]
        total = np.zeros((128, 128), np.float64)
        for q in range(cfg.gpb):
            total += pool_outs[br * cfg.gpb + q].astype(np.float64)
        gid = host_meta[sfx]["gid"]
        cnt = np.bincount(gid, minlength=128).astype(np.float64)
        total += cnt[:, None] * host_meta[sfx]["blpn"].astype(np.float64)[None, :]
        out[sfx] = (total / np.maximum(cnt[:, None], 1.0))[:cfg.G]
    cat = np.concatenate([out["A"], out["B"]], axis=1)
    Wo = np.asarray(inputs["Wo"]).astype(np.float64)
    bo = np.asarray(inputs["bo"]).astype(np.float64)
    return (cat @ Wo + bo).astype(np.float64)


_CACHE = {}


def kernel(**inputs):
    cfg = Cfg(N=inputs["featsA"].shape[0], G=128)
    nc_b, in_maps, host_meta = _prepare(cfg, inputs)
    key = ("prog", tuple(nc_b.tolist()))
    if key not in _CACHE:
        _CACHE[key] = build_program(cfg, nc_b)
    nc = _CACHE[key]
    from concourse.bass_utils import run_bass_kernel_spmd
    res = run_bass_kernel_spmd(nc, in_maps, list(range(cfg.n_cores)))
    pool_outs = [r["pool_out"] for r in res.results]
    return _finalize(cfg, inputs, host_meta, pool_outs)
